# revision 12
# baseline (speedup 1.0000x reference)
"""Causal self-attention Trainium2 kernel (8-core SPMD), v2.

Problem: x[2,2048,1024], causal mask, Wqkv[3072,1024], Wo[1024,1024], fp32.
  qkv = x @ Wqkv.T ; per-head causal softmax attention ; out = attn @ Wo.T

Sharding (data + tensor parallel over heads):
  core c -> batch b = c // 4, heads {4g..4g+3} with g = c % 4.
  Each core computes Q,K,V for its 4 heads, runs causal attention, applies the
  matching 256 columns of Wo, and writes a partial [2048,1024] output; the
  host sums the 4 partials per batch.

v2 changes vs v1 (trace-driven):
  - Projections emitted as compact 8-matmul chains into 1-bank PSUM tiles and
    woven between attention score-pairs as PE filler, so the PE never idles
    while ACT exp (the slowest per-block stage, ~1.1us per 1024 cols) runs.
  - Output projection for q-chunk qc emitted as filler during later windows
    instead of serially at the end (smaller tail).
  - DMA issued in first-needed order ([128,512] x pieces, qc-major) across 4
    engine queues, so the first projection chain starts ~6us earlier.
  - Normalization reciprocal reads the AV PSUM row directly (one less copy).
  - All-bf16 compute (fp8 rejected: DR matmuls gave no net win at equal
    column rate and the error budget was too tight).
"""

import numpy as np

S = 2048
D = 1024
DH = 64
B = 2
NCORES = 8
HPC = 4  # heads per core
QKC = 2 * HPC * DH  # 512 q+k projection columns per core
VC = HPC * DH  # 256 v columns per core
P = 128
KO = D // P  # 8 contraction tiles
NQ = S // 512  # 4 q-chunks of 512

_cache = {}


def _build():
    import concourse.bacc as bacc
    import concourse.mybir as mybir
    import concourse.tile as tile

    F32 = mybir.dt.float32
    CDT = mybir.dt.bfloat16
    EXP = mybir.ActivationFunctionType.Exp

    nc = bacc.Bacc()
    xT_d = nc.dram_tensor("xT", [D, S], CDT, kind="ExternalInput")
    wqkT_d = nc.dram_tensor("wqkT", [D, QKC], CDT, kind="ExternalInput")
    wvT_d = nc.dram_tensor("wvT", [D, VC], CDT, kind="ExternalInput")
    woT_d = nc.dram_tensor("woT", [VC, D], CDT, kind="ExternalInput")
    maskT_d = nc.dram_tensor("maskT", [P, P], CDT, kind="ExternalInput")
    out_d = nc.dram_tensor("out", [S, D], F32, kind="ExternalOutput")

    with tile.TileContext(nc) as tc:
        with (
            tc.tile_pool(name="persist", bufs=1) as persist,
            tc.tile_pool(name="sb_small", bufs=4) as sb_small,
            tc.tile_pool(name="sb_exp", bufs=6) as sb_exp,
            tc.tile_pool(name="sb_out", bufs=4) as sb_out,
            tc.tile_pool(name="pp_s2", bufs=2, space="PSUM") as pp_s2,
            tc.tile_pool(name="pp_ch", bufs=2, space="PSUM") as pp_ch,
            tc.tile_pool(name="pp_av", bufs=2, space="PSUM") as pp_av,
        ):
            xT_sb = persist.tile([P, KO, S], CDT, tag="xT")
            wqkT_sb = persist.tile([P, KO, QKC], CDT, tag="wqkT")
            wvT_sb = persist.tile([P, KO, VC], CDT, tag="wvT")
            woT_sb = persist.tile([P, 2, D], CDT, tag="woT")
            maskT_sb = persist.tile([P, P], CDT, tag="maskT")
            qkT_sb = persist.tile([P, 4, S], CDT, tag="qkT")
            v_sb = persist.tile([P, 4 * NQ, HPC, DH + 1], CDT, tag="v")
            attn_sb = persist.tile([P, 2, S], CDT, tag="attn")

            # --- input DMAs: [128,512] pieces in first-needed order, 3 queues ---
            qs = [nc.sync, nc.gpsimd, nc.scalar]
            di = 0

            def dq():
                nonlocal di
                e = qs[di % 3]
                di += 1
                return e

            for ko in range(KO):
                dq().dma_start(wqkT_sb[:, ko, :], wqkT_d[ko * P : (ko + 1) * P, :])
            for ko in range(KO):
                dq().dma_start(
                    xT_sb[:, ko, 0:512], xT_d[ko * P : (ko + 1) * P, 0:512]
                )
            for ko in range(KO):
                dq().dma_start(wvT_sb[:, ko, :], wvT_d[ko * P : (ko + 1) * P, :])
            dq().dma_start(maskT_sb[:], maskT_d[:])
            for qc in (1, 2):
                for ko in range(KO):
                    dq().dma_start(
                        xT_sb[:, ko, qc * 512 : (qc + 1) * 512],
                        xT_d[ko * P : (ko + 1) * P, qc * 512 : (qc + 1) * 512],
                    )
            dq().dma_start(woT_sb[:], woT_d.rearrange("(ct p) e -> p ct e", p=P))
            for ko in range(KO):
                dq().dma_start(
                    xT_sb[:, ko, 3 * 512 : 4 * 512],
                    xT_d[ko * P : (ko + 1) * P, 3 * 512 : 4 * 512],
                )

            ones_f32 = sb_small.tile([P, DH], F32, tag="ones")
            nc.vector.memset(ones_f32[:], 1.0)
            nc.vector.tensor_copy(
                out=v_sb[:, :, :, DH],
                in_=ones_f32[:, 0 : 4 * NQ * HPC].rearrange(
                    "p (a b) -> p a b", a=4 * NQ
                ),
            )

            # ---------- filler units (PE work with no ACT dependency) ----------
            def qk_chain(qc, slot):
                ch = pp_ch.tile([P, 512], F32, tag="ch")
                for ko in range(KO):
                    nc.tensor.matmul(
                        ch[:],
                        wqkT_sb[:, ko, slot * P : (slot + 1) * P],
                        xT_sb[:, ko, qc * 512 : (qc + 1) * 512],
                        start=(ko == 0),
                        stop=(ko == KO - 1),
                        skip_group_check=True,
                    )
                nc.vector.tensor_copy(
                    out=qkT_sb[:, slot, qc * 512 : (qc + 1) * 512], in_=ch[:]
                )

            def v_chain(qc, j):
                sc = 4 * qc + j
                ch = pp_ch.tile([P, 512], F32, tag="ch")
                for ko in range(KO):
                    nc.tensor.matmul(
                        ch[:, 0:VC],
                        xT_sb[:, ko, sc * P : (sc + 1) * P],
                        wvT_sb[:, ko, :],
                        start=(ko == 0),
                        stop=(ko == KO - 1),
                        skip_group_check=True,
                    )
                nc.vector.tensor_copy(
                    out=v_sb[:, sc, :, 0:DH],
                    in_=ch[:, 0:VC].rearrange("p (h d) -> p h d", h=HPC),
                )

            def outproj_unit(sc, en, tail=False):
                ps_o = pp_ch.tile([P, 512], F32, tag="ch")
                for ct in range(2):
                    nc.tensor.matmul(
                        ps_o[:],
                        attn_sb[:, ct, sc * P : (sc + 1) * P],
                        woT_sb[:, ct, en * 512 : (en + 1) * 512],
                        start=(ct == 0),
                        stop=(ct == 1),
                        skip_group_check=True,
                    )
                o_sb = sb_out.tile([P, 512], F32, tag="osb")
                if tail and en == 1:
                    nc.scalar.copy(out=o_sb[:], in_=ps_o[:])
                else:
                    nc.vector.tensor_copy(out=o_sb[:], in_=ps_o[:])
                dq().dma_start(
                    out_d[sc * P : (sc + 1) * P, en * 512 : (en + 1) * 512],
                    o_sb[:],
                )

            # ---------- attention for one (qc, head), weaving fillers ----------
            def attention_head(qc, h, fillers, post):
                hp = (h % 2) * DH
                mq = h // 2
                nkb = 4 * qc + 4
                avs = []
                pair_idx = 0
                for kb0 in range(0, nkb, 2):
                    ps2 = pp_s2.tile([P, 1024], F32, tag="s2")
                    exp2 = sb_exp.tile([P, 1024], CDT, tag="exp")
                    offs = []
                    for half in (0, 1):
                        kb = kb0 + half
                        m = kb - 4 * qc
                        off = max(0, m) * P
                        offs.append(off)
                        nc.tensor.matmul(
                            ps2[:, half * 512 + off : half * 512 + 512],
                            qkT_sb[hp : hp + DH, 2 + mq, kb * P : (kb + 1) * P],
                            qkT_sb[
                                hp : hp + DH, mq, qc * 512 + off : (qc + 1) * 512
                            ],
                            start=True,
                            stop=True,
                            skip_group_check=True,
                        )
                    if offs[0] == 0 and offs[1] == 0:
                        nc.scalar.activation(exp2[:], ps2[:], EXP, scale=0.125)
                    else:
                        for half, off in enumerate(offs):
                            lo = half * 512 + off
                            nc.scalar.activation(
                                exp2[:, lo : half * 512 + 512],
                                ps2[:, lo : half * 512 + 512],
                                EXP,
                                scale=0.125,
                            )
                    for half, off in enumerate(offs):
                        kb = kb0 + half
                        if kb - 4 * qc >= 0:
                            lo = half * 512 + off
                            nc.vector.tensor_mul(
                                out=exp2[:, lo : lo + P],
                                in0=exp2[:, lo : lo + P],
                                in1=maskT_sb[:],
                            )
                        avs.append((exp2, half * 512 + off, off, kb))
                    pair_idx += 1
                    if pair_idx % 2 == 0 and fillers:
                        fillers.popleft()()
                ps_av = pp_av.tile([DH + 1, 512], F32, tag="av")
                for j, (exp2, lo, off, kb) in enumerate(avs):
                    nc.tensor.matmul(
                        ps_av[:, off:512],
                        v_sb[:, kb, h, :],
                        exp2[:, lo : (lo - off) + 512],
                        start=(j == 0),
                        stop=(j == len(avs) - 1),
                        skip_group_check=True,
                    )
                sums_sb = sb_small.tile([1, 512], F32, tag="sums")
                nc.vector.tensor_copy(out=sums_sb[:], in_=ps_av[DH : DH + 1, :])
                recip_f = sb_small.tile([1, 512], F32, tag="recipf")
                nc.vector.reciprocal_approx_fast(out=recip_f[:], in_=sums_sb[:])
                bc_sb = sb_small.tile([DH, 512], F32, tag="bc")
                nc.gpsimd.partition_broadcast(bc_sb[:], recip_f[:])
                nc.vector.tensor_mul(
                    out=attn_sb[hp : hp + DH, mq, qc * 512 : (qc + 1) * 512],
                    in0=ps_av[0:DH, :],
                    in1=bc_sb[:],
                )
                for _ in range(post):
                    if fillers:
                        fillers.popleft()()

            # ---------- schedule ----------
            from collections import deque

            # bootstrap: projections for qc0 (slot 0 first: its weights arrive first)
            for slot in (0, 2, 1, 3):
                qk_chain(0, slot)
            for j in range(4):
                v_chain(0, j)

            # window fillers: PE work without ACT deps, woven into attention
            win = {
                0: [lambda s=s: qk_chain(1, s) for s in range(4)]
                + [lambda j=j: v_chain(1, j) for j in range(4)],
                1: [lambda s=s: qk_chain(2, s) for s in range(4)]
                + [lambda j=j: v_chain(2, j) for j in range(4)]
                + [lambda sc=sc, en=en: outproj_unit(sc, en)
                   for sc in range(4) for en in range(2)],
                2: [lambda s=s: qk_chain(3, s) for s in range(4)]
                + [lambda sc=sc, en=en: outproj_unit(sc, en)
                   for sc in range(4, 8) for en in range(2)][:4],
                3: [lambda j=j: v_chain(3, j) for j in range(4)]
                + [lambda sc=sc, en=en: outproj_unit(sc, en)
                   for sc in range(4, 8) for en in range(2)][4:]
                + [lambda sc=sc, en=en: outproj_unit(sc, en)
                   for sc in range(8, 12) for en in range(2)],
            }
            post = {0: 1, 1: 2, 2: 1, 3: 1}
            for qc in range(NQ):
                fillers = deque(win[qc])
                for h in range(HPC):
                    attention_head(qc, h, fillers, post[qc])
                while fillers:
                    fillers.popleft()()
            # tail: outproj for q-chunks 12..15
            for sc in range(12, 16):
                for en in range(2):
                    outproj_unit(sc, en, tail=True)

    nc.compile()
    return nc


def _get_nc():
    if "nc" not in _cache:
        _cache["nc"] = _build()
    return _cache["nc"]


def _shard(x, mask, Wqkv, Wo):
    import ml_dtypes

    cdt = ml_dtypes.bfloat16
    in_maps = []
    maskT = np.ascontiguousarray((mask[0, 0, :P, :P].T >= 0).astype(cdt))
    for c in range(NCORES):
        b = c // 4
        g = c % 4
        heads = [4 * g + i for i in range(HPC)]
        q_rows = np.concatenate([np.arange(h * DH, (h + 1) * DH) for h in heads])
        k_rows = D + q_rows
        v_rows = 2 * D + q_rows
        qk_rows = np.concatenate([q_rows, k_rows])
        in_maps.append(
            {
                "xT": np.ascontiguousarray(x[b].T.astype(cdt)),
                "wqkT": np.ascontiguousarray(Wqkv[qk_rows, :].T.astype(cdt)),
                "wvT": np.ascontiguousarray(Wqkv[v_rows, :].T.astype(cdt)),
                "woT": np.ascontiguousarray(Wo[:, q_rows].T.astype(cdt)),
                "maskT": maskT,
            }
        )
    return in_maps


def kernel(x, mask, Wqkv, Wo, _trace=False):
    from concourse.bass_utils import run_bass_kernel_spmd

    x = np.asarray(x, dtype=np.float32)
    mask = np.asarray(mask, dtype=np.float32)
    Wqkv = np.asarray(Wqkv, dtype=np.float32)
    Wo = np.asarray(Wo, dtype=np.float32)

    nc = _get_nc()
    in_maps = _shard(x, mask, Wqkv, Wo)
    res = run_bass_kernel_spmd(nc, in_maps, core_ids=list(range(NCORES)), trace=_trace)
    _cache["last_result"] = res

    out = np.zeros((B, S, D), dtype=np.float32)
    for c in range(NCORES):
        out[c // 4] += np.asarray(res.results[c]["out"], dtype=np.float32)
    return out


# revision 13
# speedup vs baseline: 1.0015x; 1.0015x over previous
"""Causal self-attention Trainium2 kernel (8-core SPMD), v2.

Problem: x[2,2048,1024], causal mask, Wqkv[3072,1024], Wo[1024,1024], fp32.
  qkv = x @ Wqkv.T ; per-head causal softmax attention ; out = attn @ Wo.T

Sharding (data + tensor parallel over heads):
  core c -> batch b = c // 4, heads {4g..4g+3} with g = c % 4.
  Each core computes Q,K,V for its 4 heads, runs causal attention, applies the
  matching 256 columns of Wo, and writes a partial [2048,1024] output; the
  host sums the 4 partials per batch.

v2 changes vs v1 (trace-driven):
  - Projections emitted as compact 8-matmul chains into 1-bank PSUM tiles and
    woven between attention score-pairs as PE filler, so the PE never idles
    while ACT exp (the slowest per-block stage, ~1.1us per 1024 cols) runs.
  - Output projection for q-chunk qc emitted as filler during later windows
    instead of serially at the end (smaller tail).
  - DMA issued in first-needed order ([128,512] x pieces, qc-major) across 4
    engine queues, so the first projection chain starts ~6us earlier.
  - Normalization reciprocal reads the AV PSUM row directly (one less copy).
  - All-bf16 compute (fp8 rejected: DR matmuls gave no net win at equal
    column rate and the error budget was too tight).
"""

import numpy as np

S = 2048
D = 1024
DH = 64
B = 2
NCORES = 8
HPC = 4  # heads per core
QKC = 2 * HPC * DH  # 512 q+k projection columns per core
VC = HPC * DH  # 256 v columns per core
P = 128
KO = D // P  # 8 contraction tiles
NQ = S // 512  # 4 q-chunks of 512

_cache = {}


def _build():
    import concourse.bacc as bacc
    import concourse.mybir as mybir
    import concourse.tile as tile

    F32 = mybir.dt.float32
    CDT = mybir.dt.bfloat16
    EXP = mybir.ActivationFunctionType.Exp

    nc = bacc.Bacc()
    xT_d = nc.dram_tensor("xT", [D, S], CDT, kind="ExternalInput")
    wqkT_d = nc.dram_tensor("wqkT", [D, QKC], CDT, kind="ExternalInput")
    wvT_d = nc.dram_tensor("wvT", [D, VC], CDT, kind="ExternalInput")
    woT_d = nc.dram_tensor("woT", [VC, D], CDT, kind="ExternalInput")
    maskT_d = nc.dram_tensor("maskT", [P, P], CDT, kind="ExternalInput")
    out_d = nc.dram_tensor("out", [S, D], F32, kind="ExternalOutput")

    with tile.TileContext(nc) as tc:
        with (
            tc.tile_pool(name="persist", bufs=1) as persist,
            tc.tile_pool(name="sb_small", bufs=4) as sb_small,
            tc.tile_pool(name="sb_exp", bufs=6) as sb_exp,
            tc.tile_pool(name="sb_out", bufs=4) as sb_out,
            tc.tile_pool(name="pp_s2", bufs=2, space="PSUM") as pp_s2,
            tc.tile_pool(name="pp_ch", bufs=2, space="PSUM") as pp_ch,
            tc.tile_pool(name="pp_av", bufs=2, space="PSUM") as pp_av,
        ):
            xT_sb = persist.tile([P, KO, S], CDT, tag="xT")
            wqkT_sb = persist.tile([P, KO, QKC], CDT, tag="wqkT")
            wvT_sb = persist.tile([P, KO, VC], CDT, tag="wvT")
            woT_sb = persist.tile([P, 2, D], CDT, tag="woT")
            maskT_sb = persist.tile([P, P], CDT, tag="maskT")
            qkT_sb = persist.tile([P, 4, S], CDT, tag="qkT")
            v_sb = persist.tile([P, 4 * NQ, HPC, DH + 1], CDT, tag="v")
            attn_sb = persist.tile([P, 2, S], CDT, tag="attn")

            # --- input DMAs: [128,512] pieces in first-needed order, 3 queues ---
            qs = [nc.sync, nc.gpsimd, nc.scalar]
            di = 0

            def dq():
                nonlocal di
                e = qs[di % 3]
                di += 1
                return e

            for ko in range(KO):
                dq().dma_start(wqkT_sb[:, ko, :], wqkT_d[ko * P : (ko + 1) * P, :])
            for ko in range(KO):
                dq().dma_start(
                    xT_sb[:, ko, 0:512], xT_d[ko * P : (ko + 1) * P, 0:512]
                )
            for ko in range(KO):
                dq().dma_start(wvT_sb[:, ko, :], wvT_d[ko * P : (ko + 1) * P, :])
            dq().dma_start(maskT_sb[:], maskT_d[:])
            for qc in (1, 2):
                for ko in range(KO):
                    dq().dma_start(
                        xT_sb[:, ko, qc * 512 : (qc + 1) * 512],
                        xT_d[ko * P : (ko + 1) * P, qc * 512 : (qc + 1) * 512],
                    )
            dq().dma_start(woT_sb[:], woT_d.rearrange("(ct p) e -> p ct e", p=P))
            for ko in range(KO):
                dq().dma_start(
                    xT_sb[:, ko, 3 * 512 : 4 * 512],
                    xT_d[ko * P : (ko + 1) * P, 3 * 512 : 4 * 512],
                )

            ones_f32 = sb_small.tile([P, DH], F32, tag="ones")
            nc.vector.memset(ones_f32[:], 1.0)
            nc.vector.tensor_copy(
                out=v_sb[:, :, :, DH],
                in_=ones_f32[:, 0 : 4 * NQ * HPC].rearrange(
                    "p (a b) -> p a b", a=4 * NQ
                ),
            )

            # ---------- filler units (PE work with no ACT dependency) ----------
            def qk_chain(qc, slot):
                ch = pp_ch.tile([P, 512], F32, tag="ch")
                for ko in range(KO):
                    nc.tensor.matmul(
                        ch[:],
                        wqkT_sb[:, ko, slot * P : (slot + 1) * P],
                        xT_sb[:, ko, qc * 512 : (qc + 1) * 512],
                        start=(ko == 0),
                        stop=(ko == KO - 1),
                        skip_group_check=True,
                    )
                nc.vector.tensor_copy(
                    out=qkT_sb[:, slot, qc * 512 : (qc + 1) * 512], in_=ch[:]
                )

            def v_chain(qc, j):
                sc = 4 * qc + j
                ch = pp_ch.tile([P, 512], F32, tag="ch")
                for ko in range(KO):
                    nc.tensor.matmul(
                        ch[:, 0:VC],
                        xT_sb[:, ko, sc * P : (sc + 1) * P],
                        wvT_sb[:, ko, :],
                        start=(ko == 0),
                        stop=(ko == KO - 1),
                        skip_group_check=True,
                    )
                nc.vector.tensor_copy(
                    out=v_sb[:, sc, :, 0:DH],
                    in_=ch[:, 0:VC].rearrange("p (h d) -> p h d", h=HPC),
                )

            def outproj_unit(sc, en, tail=False):
                ps_o = pp_ch.tile([P, 512], F32, tag="ch")
                for ct in range(2):
                    nc.tensor.matmul(
                        ps_o[:],
                        attn_sb[:, ct, sc * P : (sc + 1) * P],
                        woT_sb[:, ct, en * 512 : (en + 1) * 512],
                        start=(ct == 0),
                        stop=(ct == 1),
                        skip_group_check=True,
                    )
                o_sb = sb_out.tile([P, 512], F32, tag="osb")
                if tail and en == 1:
                    nc.scalar.copy(out=o_sb[:], in_=ps_o[:])
                else:
                    nc.vector.tensor_copy(out=o_sb[:], in_=ps_o[:])
                (nc.gpsimd if tail else nc.sync).dma_start(
                    out_d[sc * P : (sc + 1) * P, en * 512 : (en + 1) * 512],
                    o_sb[:],
                )

            # ---------- attention for one (qc, head), weaving fillers ----------
            def attention_head(qc, h, fillers, post):
                hp = (h % 2) * DH
                mq = h // 2
                nkb = 4 * qc + 4
                avs = []
                pair_idx = 0
                for kb0 in range(0, nkb, 2):
                    ps2 = pp_s2.tile([P, 1024], F32, tag="s2")
                    exp2 = sb_exp.tile([P, 1024], CDT, tag="exp")
                    offs = []
                    for half in (0, 1):
                        kb = kb0 + half
                        m = kb - 4 * qc
                        off = max(0, m) * P
                        offs.append(off)
                        nc.tensor.matmul(
                            ps2[:, half * 512 + off : half * 512 + 512],
                            qkT_sb[hp : hp + DH, 2 + mq, kb * P : (kb + 1) * P],
                            qkT_sb[
                                hp : hp + DH, mq, qc * 512 + off : (qc + 1) * 512
                            ],
                            start=True,
                            stop=True,
                            skip_group_check=True,
                        )
                    if offs[0] == 0 and offs[1] == 0:
                        nc.scalar.activation(exp2[:], ps2[:], EXP, scale=0.125)
                    else:
                        for half, off in enumerate(offs):
                            lo = half * 512 + off
                            nc.scalar.activation(
                                exp2[:, lo : half * 512 + 512],
                                ps2[:, lo : half * 512 + 512],
                                EXP,
                                scale=0.125,
                            )
                    for half, off in enumerate(offs):
                        kb = kb0 + half
                        if kb - 4 * qc >= 0:
                            lo = half * 512 + off
                            nc.vector.tensor_mul(
                                out=exp2[:, lo : lo + P],
                                in0=exp2[:, lo : lo + P],
                                in1=maskT_sb[:],
                            )
                        avs.append((exp2, half * 512 + off, off, kb))
                    pair_idx += 1
                    if pair_idx % 2 == 0 and fillers:
                        fillers.popleft()()
                ps_av = pp_av.tile([DH + 1, 512], F32, tag="av")
                for j, (exp2, lo, off, kb) in enumerate(avs):
                    nc.tensor.matmul(
                        ps_av[:, off:512],
                        v_sb[:, kb, h, :],
                        exp2[:, lo : (lo - off) + 512],
                        start=(j == 0),
                        stop=(j == len(avs) - 1),
                        skip_group_check=True,
                    )
                sums_sb = sb_small.tile([1, 512], F32, tag="sums")
                nc.vector.tensor_copy(out=sums_sb[:], in_=ps_av[DH : DH + 1, :])
                recip_f = sb_small.tile([1, 512], F32, tag="recipf")
                nc.vector.reciprocal_approx_fast(out=recip_f[:], in_=sums_sb[:])
                bc_sb = sb_small.tile([DH, 512], F32, tag="bc")
                nc.gpsimd.partition_broadcast(bc_sb[:], recip_f[:])
                nc.vector.tensor_mul(
                    out=attn_sb[hp : hp + DH, mq, qc * 512 : (qc + 1) * 512],
                    in0=ps_av[0:DH, :],
                    in1=bc_sb[:],
                )
                for _ in range(post):
                    if fillers:
                        fillers.popleft()()

            # ---------- schedule ----------
            from collections import deque

            # bootstrap: projections for qc0 (slot 0 first: its weights arrive first)
            for slot in (0, 2, 1, 3):
                qk_chain(0, slot)
            for j in range(4):
                v_chain(0, j)

            # window fillers: PE work without ACT deps, woven into attention
            win = {
                0: [lambda s=s: qk_chain(1, s) for s in range(4)]
                + [lambda j=j: v_chain(1, j) for j in range(4)],
                1: [lambda s=s: qk_chain(2, s) for s in range(4)]
                + [lambda j=j: v_chain(2, j) for j in range(4)]
                + [lambda sc=sc, en=en: outproj_unit(sc, en)
                   for sc in range(4) for en in range(2)],
                2: [lambda s=s: qk_chain(3, s) for s in range(4)]
                + [lambda sc=sc, en=en: outproj_unit(sc, en)
                   for sc in range(4, 8) for en in range(2)][:4],
                3: [lambda j=j: v_chain(3, j) for j in range(4)]
                + [lambda sc=sc, en=en: outproj_unit(sc, en)
                   for sc in range(4, 8) for en in range(2)][4:]
                + [lambda sc=sc, en=en: outproj_unit(sc, en)
                   for sc in range(8, 12) for en in range(2)],
            }
            post = {0: 1, 1: 2, 2: 1, 3: 1}
            for qc in range(NQ):
                fillers = deque(win[qc])
                for h in range(HPC):
                    attention_head(qc, h, fillers, post[qc])
                while fillers:
                    fillers.popleft()()
            # tail: outproj for q-chunks 12..15
            for sc in range(12, 16):
                for en in range(2):
                    outproj_unit(sc, en, tail=True)

    nc.compile()
    return nc


def _get_nc():
    if "nc" not in _cache:
        _cache["nc"] = _build()
    return _cache["nc"]


def _shard(x, mask, Wqkv, Wo):
    import ml_dtypes

    cdt = ml_dtypes.bfloat16
    in_maps = []
    maskT = np.ascontiguousarray((mask[0, 0, :P, :P].T >= 0).astype(cdt))
    for c in range(NCORES):
        b = c // 4
        g = c % 4
        heads = [4 * g + i for i in range(HPC)]
        q_rows = np.concatenate([np.arange(h * DH, (h + 1) * DH) for h in heads])
        k_rows = D + q_rows
        v_rows = 2 * D + q_rows
        qk_rows = np.concatenate([q_rows, k_rows])
        in_maps.append(
            {
                "xT": np.ascontiguousarray(x[b].T.astype(cdt)),
                "wqkT": np.ascontiguousarray(Wqkv[qk_rows, :].T.astype(cdt)),
                "wvT": np.ascontiguousarray(Wqkv[v_rows, :].T.astype(cdt)),
                "woT": np.ascontiguousarray(Wo[:, q_rows].T.astype(cdt)),
                "maskT": maskT,
            }
        )
    return in_maps


def kernel(x, mask, Wqkv, Wo, _trace=False):
    from concourse.bass_utils import run_bass_kernel_spmd

    x = np.asarray(x, dtype=np.float32)
    mask = np.asarray(mask, dtype=np.float32)
    Wqkv = np.asarray(Wqkv, dtype=np.float32)
    Wo = np.asarray(Wo, dtype=np.float32)

    nc = _get_nc()
    in_maps = _shard(x, mask, Wqkv, Wo)
    res = run_bass_kernel_spmd(nc, in_maps, core_ids=list(range(NCORES)), trace=_trace)
    _cache["last_result"] = res

    out = np.zeros((B, S, D), dtype=np.float32)
    for c in range(NCORES):
        out[c // 4] += np.asarray(res.results[c]["out"], dtype=np.float32)
    return out


# revision 14
# speedup vs baseline: 1.0039x; 1.0024x over previous
"""Causal self-attention Trainium2 kernel (8-core SPMD), v2.

Problem: x[2,2048,1024], causal mask, Wqkv[3072,1024], Wo[1024,1024], fp32.
  qkv = x @ Wqkv.T ; per-head causal softmax attention ; out = attn @ Wo.T

Sharding (data + tensor parallel over heads):
  core c -> batch b = c // 4, heads {4g..4g+3} with g = c % 4.
  Each core computes Q,K,V for its 4 heads, runs causal attention, applies the
  matching 256 columns of Wo, and writes a partial [2048,1024] output; the
  host sums the 4 partials per batch.

v2 changes vs v1 (trace-driven):
  - Projections emitted as compact 8-matmul chains into 1-bank PSUM tiles and
    woven between attention score-pairs as PE filler, so the PE never idles
    while ACT exp (the slowest per-block stage, ~1.1us per 1024 cols) runs.
  - Output projection for q-chunk qc emitted as filler during later windows
    instead of serially at the end (smaller tail).
  - DMA issued in first-needed order ([128,512] x pieces, qc-major) across 4
    engine queues, so the first projection chain starts ~6us earlier.
  - Normalization reciprocal reads the AV PSUM row directly (one less copy).
  - All-bf16 compute (fp8 rejected: DR matmuls gave no net win at equal
    column rate and the error budget was too tight).
"""

import numpy as np

S = 2048
D = 1024
DH = 64
B = 2
NCORES = 8
HPC = 4  # heads per core
QKC = 2 * HPC * DH  # 512 q+k projection columns per core
VC = HPC * DH  # 256 v columns per core
P = 128
KO = D // P  # 8 contraction tiles
NQ = S // 512  # 4 q-chunks of 512

_cache = {}


def _build():
    import concourse.bacc as bacc
    import concourse.mybir as mybir
    import concourse.tile as tile

    F32 = mybir.dt.float32
    CDT = mybir.dt.bfloat16
    EXP = mybir.ActivationFunctionType.Exp

    nc = bacc.Bacc()
    xT_d = nc.dram_tensor("xT", [D, S], CDT, kind="ExternalInput")
    wqkT_d = nc.dram_tensor("wqkT", [D, QKC], CDT, kind="ExternalInput")
    wvT_d = nc.dram_tensor("wvT", [D, VC], CDT, kind="ExternalInput")
    woT_d = nc.dram_tensor("woT", [VC, D], CDT, kind="ExternalInput")
    maskT_d = nc.dram_tensor("maskT", [P, P], CDT, kind="ExternalInput")
    out_d = nc.dram_tensor("out", [S, D], F32, kind="ExternalOutput")

    with tile.TileContext(nc) as tc:
        with (
            tc.tile_pool(name="persist", bufs=1) as persist,
            tc.tile_pool(name="sb_small", bufs=4) as sb_small,
            tc.tile_pool(name="sb_exp", bufs=6) as sb_exp,
            tc.tile_pool(name="sb_out", bufs=4) as sb_out,
            tc.tile_pool(name="pp_s2", bufs=2, space="PSUM") as pp_s2,
            tc.tile_pool(name="pp_ch", bufs=2, space="PSUM") as pp_ch,
            tc.tile_pool(name="pp_av", bufs=2, space="PSUM") as pp_av,
        ):
            xT_sb = persist.tile([P, KO, S], CDT, tag="xT")
            wqkT_sb = persist.tile([P, KO, QKC], CDT, tag="wqkT")
            wvT_sb = persist.tile([P, KO, VC], CDT, tag="wvT")
            woT_sb = persist.tile([P, 2, D], CDT, tag="woT")
            maskT_sb = persist.tile([P, P], CDT, tag="maskT")
            qkT_sb = persist.tile([P, 4, S], CDT, tag="qkT")
            v_sb = persist.tile([P, 4 * NQ, HPC, DH + 1], CDT, tag="v")
            attn_sb = persist.tile([P, 2, S], CDT, tag="attn")

            # --- input DMAs: [128,512] pieces in first-needed order, 3 queues ---
            qs = [nc.sync, nc.gpsimd, nc.scalar]
            di = 0

            def dq():
                nonlocal di
                e = qs[di % 3]
                di += 1
                return e

            for ko in range(KO):
                dq().dma_start(wqkT_sb[:, ko, :], wqkT_d[ko * P : (ko + 1) * P, :])
            for ko in range(KO):
                dq().dma_start(
                    xT_sb[:, ko, 0:512], xT_d[ko * P : (ko + 1) * P, 0:512]
                )
            for ko in range(KO):
                dq().dma_start(wvT_sb[:, ko, :], wvT_d[ko * P : (ko + 1) * P, :])
            dq().dma_start(maskT_sb[:], maskT_d[:])
            for qc in (1, 2):
                for ko in range(KO):
                    dq().dma_start(
                        xT_sb[:, ko, qc * 512 : (qc + 1) * 512],
                        xT_d[ko * P : (ko + 1) * P, qc * 512 : (qc + 1) * 512],
                    )
            dq().dma_start(woT_sb[:], woT_d.rearrange("(ct p) e -> p ct e", p=P))
            for ko in range(KO):
                dq().dma_start(
                    xT_sb[:, ko, 3 * 512 : 4 * 512],
                    xT_d[ko * P : (ko + 1) * P, 3 * 512 : 4 * 512],
                )

            ones_f32 = sb_small.tile([P, DH], F32, tag="ones")
            nc.vector.memset(ones_f32[:], 1.0)
            nc.vector.tensor_copy(
                out=v_sb[:, :, :, DH],
                in_=ones_f32[:, 0 : 4 * NQ * HPC].rearrange(
                    "p (a b) -> p a b", a=4 * NQ
                ),
            )

            # ---------- filler units (PE work with no ACT dependency) ----------
            def qk_chain(qc, slot):
                ch = pp_ch.tile([P, 512], F32, tag="ch")
                for ko in range(KO):
                    nc.tensor.matmul(
                        ch[:],
                        wqkT_sb[:, ko, slot * P : (slot + 1) * P],
                        xT_sb[:, ko, qc * 512 : (qc + 1) * 512],
                        start=(ko == 0),
                        stop=(ko == KO - 1),
                        skip_group_check=True,
                    )
                nc.vector.tensor_copy(
                    out=qkT_sb[:, slot, qc * 512 : (qc + 1) * 512], in_=ch[:]
                )

            def v_chain(qc, j):
                sc = 4 * qc + j
                ch = pp_ch.tile([P, 512], F32, tag="ch")
                for ko in range(KO):
                    nc.tensor.matmul(
                        ch[:, 0:VC],
                        xT_sb[:, ko, sc * P : (sc + 1) * P],
                        wvT_sb[:, ko, :],
                        start=(ko == 0),
                        stop=(ko == KO - 1),
                        skip_group_check=True,
                    )
                nc.vector.tensor_copy(
                    out=v_sb[:, sc, :, 0:DH],
                    in_=ch[:, 0:VC].rearrange("p (h d) -> p h d", h=HPC),
                )

            def outproj_unit(sc, en, tail=False):
                ps_o = pp_ch.tile([P, 512], F32, tag="ch")
                for ct in range(2):
                    nc.tensor.matmul(
                        ps_o[:],
                        attn_sb[:, ct, sc * P : (sc + 1) * P],
                        woT_sb[:, ct, en * 512 : (en + 1) * 512],
                        start=(ct == 0),
                        stop=(ct == 1),
                        skip_group_check=True,
                    )
                o_sb = sb_out.tile([P, 512], F32, tag="osb")
                if tail and en == 1:
                    nc.scalar.copy(out=o_sb[:], in_=ps_o[:])
                else:
                    nc.vector.tensor_copy(out=o_sb[:], in_=ps_o[:])
                (nc.gpsimd if tail else nc.sync).dma_start(
                    out_d[sc * P : (sc + 1) * P, en * 512 : (en + 1) * 512],
                    o_sb[:],
                )

            # ---------- attention for one (qc, head), weaving fillers ----------
            def attention_head(qc, h, fillers, post):
                hp = (h % 2) * DH
                mq = h // 2
                nkb = 4 * qc + 4
                avs = []
                pair_idx = 0
                for kb0 in range(0, nkb, 2):
                    ps2 = pp_s2.tile([P, 1024], F32, tag="s2")
                    exp2 = sb_exp.tile([P, 1024], CDT, tag="exp")
                    offs = []
                    for half in (0, 1):
                        kb = kb0 + half
                        m = kb - 4 * qc
                        off = max(0, m) * P
                        offs.append(off)
                        nc.tensor.matmul(
                            ps2[:, half * 512 + off : half * 512 + 512],
                            qkT_sb[hp : hp + DH, 2 + mq, kb * P : (kb + 1) * P],
                            qkT_sb[
                                hp : hp + DH, mq, qc * 512 + off : (qc + 1) * 512
                            ],
                            start=True,
                            stop=True,
                            skip_group_check=True,
                        )
                    if offs[0] == 0 and offs[1] == 0:
                        nc.scalar.activation(exp2[:], ps2[:], EXP, scale=0.125)
                    else:
                        for half, off in enumerate(offs):
                            lo = half * 512 + off
                            nc.scalar.activation(
                                exp2[:, lo : half * 512 + 512],
                                ps2[:, lo : half * 512 + 512],
                                EXP,
                                scale=0.125,
                            )
                    for half, off in enumerate(offs):
                        kb = kb0 + half
                        if kb - 4 * qc >= 0:
                            lo = half * 512 + off
                            nc.vector.tensor_mul(
                                out=exp2[:, lo : lo + P],
                                in0=exp2[:, lo : lo + P],
                                in1=maskT_sb[:],
                            )
                        avs.append((exp2, half * 512 + off, off, kb))
                    pair_idx += 1
                    if pair_idx % 2 == 0 and fillers:
                        fillers.popleft()()
                ps_av = pp_av.tile([DH + 1, 512], F32, tag="av")
                for j, (exp2, lo, off, kb) in enumerate(avs):
                    nc.tensor.matmul(
                        ps_av[:, off:512],
                        v_sb[:, kb, h, :],
                        exp2[:, lo : (lo - off) + 512],
                        start=(j == 0),
                        stop=(j == len(avs) - 1),
                        skip_group_check=True,
                    )
                sums_sb = sb_small.tile([1, 512], F32, tag="sums")
                nc.vector.tensor_copy(out=sums_sb[:], in_=ps_av[DH : DH + 1, :])
                recip_f = sb_small.tile([1, 512], F32, tag="recipf")
                nc.vector.reciprocal_approx_fast(out=recip_f[:], in_=sums_sb[:])
                bc_sb = sb_small.tile([DH, 512], F32, tag="bc")
                nc.gpsimd.partition_broadcast(bc_sb[:], recip_f[:])
                nc.vector.tensor_mul(
                    out=attn_sb[hp : hp + DH, mq, qc * 512 : (qc + 1) * 512],
                    in0=ps_av[0:DH, :],
                    in1=bc_sb[:],
                )
                for _ in range(post):
                    if fillers:
                        fillers.popleft()()

            # ---------- schedule ----------
            from collections import deque

            # bootstrap: projections for qc0
            for slot in range(4):
                qk_chain(0, slot)
            for j in range(4):
                v_chain(0, j)

            # window fillers: PE work without ACT deps, woven into attention
            win = {
                0: [lambda s=s: qk_chain(1, s) for s in range(4)]
                + [lambda j=j: v_chain(1, j) for j in range(4)],
                1: [lambda s=s: qk_chain(2, s) for s in range(4)]
                + [lambda j=j: v_chain(2, j) for j in range(4)]
                + [lambda sc=sc, en=en: outproj_unit(sc, en)
                   for sc in range(4) for en in range(2)],
                2: [lambda s=s: qk_chain(3, s) for s in range(4)]
                + [lambda sc=sc, en=en: outproj_unit(sc, en)
                   for sc in range(4, 8) for en in range(2)][:4],
                3: [lambda j=j: v_chain(3, j) for j in range(4)]
                + [lambda sc=sc, en=en: outproj_unit(sc, en)
                   for sc in range(4, 8) for en in range(2)][4:]
                + [lambda sc=sc, en=en: outproj_unit(sc, en)
                   for sc in range(8, 12) for en in range(2)],
            }
            post = {0: 1, 1: 2, 2: 1, 3: 1}
            for qc in range(NQ):
                fillers = deque(win[qc])
                for h in range(HPC):
                    attention_head(qc, h, fillers, post[qc])
                while fillers:
                    fillers.popleft()()
            # tail: outproj for q-chunks 12..15
            for sc in range(12, 16):
                for en in range(2):
                    outproj_unit(sc, en, tail=True)

    nc.compile()
    return nc


def _get_nc():
    if "nc" not in _cache:
        _cache["nc"] = _build()
    return _cache["nc"]


def _shard(x, mask, Wqkv, Wo):
    import ml_dtypes

    cdt = ml_dtypes.bfloat16
    in_maps = []
    maskT = np.ascontiguousarray((mask[0, 0, :P, :P].T >= 0).astype(cdt))
    for c in range(NCORES):
        b = c // 4
        g = c % 4
        heads = [4 * g + i for i in range(HPC)]
        q_rows = np.concatenate([np.arange(h * DH, (h + 1) * DH) for h in heads])
        k_rows = D + q_rows
        v_rows = 2 * D + q_rows
        qk_rows = np.concatenate([q_rows, k_rows])
        in_maps.append(
            {
                "xT": np.ascontiguousarray(x[b].T.astype(cdt)),
                "wqkT": np.ascontiguousarray(Wqkv[qk_rows, :].T.astype(cdt)),
                "wvT": np.ascontiguousarray(Wqkv[v_rows, :].T.astype(cdt)),
                "woT": np.ascontiguousarray(Wo[:, q_rows].T.astype(cdt)),
                "maskT": maskT,
            }
        )
    return in_maps


def kernel(x, mask, Wqkv, Wo, _trace=False):
    from concourse.bass_utils import run_bass_kernel_spmd

    x = np.asarray(x, dtype=np.float32)
    mask = np.asarray(mask, dtype=np.float32)
    Wqkv = np.asarray(Wqkv, dtype=np.float32)
    Wo = np.asarray(Wo, dtype=np.float32)

    nc = _get_nc()
    in_maps = _shard(x, mask, Wqkv, Wo)
    res = run_bass_kernel_spmd(nc, in_maps, core_ids=list(range(NCORES)), trace=_trace)
    _cache["last_result"] = res

    out = np.zeros((B, S, D), dtype=np.float32)
    for c in range(NCORES):
        out[c // 4] += np.asarray(res.results[c]["out"], dtype=np.float32)
    return out


# revision 15
# speedup vs baseline: 1.0117x; 1.0078x over previous
"""Causal self-attention Trainium2 kernel (8-core SPMD), v2.

Problem: x[2,2048,1024], causal mask, Wqkv[3072,1024], Wo[1024,1024], fp32.
  qkv = x @ Wqkv.T ; per-head causal softmax attention ; out = attn @ Wo.T

Sharding (data + tensor parallel over heads):
  core c -> batch b = c // 4, heads {4g..4g+3} with g = c % 4.
  Each core computes Q,K,V for its 4 heads, runs causal attention, applies the
  matching 256 columns of Wo, and writes a partial [2048,1024] output; the
  host sums the 4 partials per batch.

v2 changes vs v1 (trace-driven):
  - Projections emitted as compact 8-matmul chains into 1-bank PSUM tiles and
    woven between attention score-pairs as PE filler, so the PE never idles
    while ACT exp (the slowest per-block stage, ~1.1us per 1024 cols) runs.
  - Output projection for q-chunk qc emitted as filler during later windows
    instead of serially at the end (smaller tail).
  - DMA issued in first-needed order ([128,512] x pieces, qc-major) across 4
    engine queues, so the first projection chain starts ~6us earlier.
  - Normalization reciprocal reads the AV PSUM row directly (one less copy).
  - All-bf16 compute (fp8 rejected: DR matmuls gave no net win at equal
    column rate and the error budget was too tight).
"""

import numpy as np

S = 2048
D = 1024
DH = 64
B = 2
NCORES = 8
HPC = 4  # heads per core
QKC = 2 * HPC * DH  # 512 q+k projection columns per core
VC = HPC * DH  # 256 v columns per core
P = 128
KO = D // P  # 8 contraction tiles
NQ = S // 512  # 4 q-chunks of 512

_cache = {}


def _build():
    import concourse.bacc as bacc
    import concourse.mybir as mybir
    import concourse.tile as tile

    F32 = mybir.dt.float32
    CDT = mybir.dt.bfloat16
    EXP = mybir.ActivationFunctionType.Exp

    nc = bacc.Bacc()
    xT_d = nc.dram_tensor("xT", [D, S], CDT, kind="ExternalInput")
    wqkT_d = nc.dram_tensor("wqkT", [D, QKC], CDT, kind="ExternalInput")
    wvT_d = nc.dram_tensor("wvT", [D, VC], CDT, kind="ExternalInput")
    woT_d = nc.dram_tensor("woT", [VC, D], CDT, kind="ExternalInput")
    maskT_d = nc.dram_tensor("maskT", [P, P], CDT, kind="ExternalInput")
    out_d = nc.dram_tensor("out", [S, D], F32, kind="ExternalOutput")

    with tile.TileContext(nc) as tc:
        with (
            tc.tile_pool(name="persist", bufs=1) as persist,
            tc.tile_pool(name="sb_small", bufs=4) as sb_small,
            tc.tile_pool(name="sb_exp", bufs=6) as sb_exp,
            tc.tile_pool(name="sb_out", bufs=4) as sb_out,
            tc.tile_pool(name="pp_s2", bufs=2, space="PSUM") as pp_s2,
            tc.tile_pool(name="pp_ch", bufs=2, space="PSUM") as pp_ch,
            tc.tile_pool(name="pp_av", bufs=2, space="PSUM") as pp_av,
        ):
            xT_sb = persist.tile([P, KO, S], CDT, tag="xT")
            wqkT_sb = persist.tile([P, KO, QKC], CDT, tag="wqkT")
            wvT_sb = persist.tile([P, KO, VC], CDT, tag="wvT")
            woT_sb = persist.tile([P, 2, D], CDT, tag="woT")
            maskT_sb = persist.tile([P, P], CDT, tag="maskT")
            qkT_sb = persist.tile([P, 4, S], CDT, tag="qkT")
            v_sb = persist.tile([P, 4 * NQ, HPC, DH + 1], CDT, tag="v")
            attn_sb = persist.tile([P, 2, S], CDT, tag="attn")

            # --- input DMAs: [128,512] pieces in first-needed order, 3 queues ---
            qs = [nc.sync, nc.gpsimd, nc.scalar]
            di = 0

            def dq():
                nonlocal di
                e = qs[di % 3]
                di += 1
                return e

            for ko in range(KO):
                dq().dma_start(wqkT_sb[:, ko, :], wqkT_d[ko * P : (ko + 1) * P, :])
            for ko in range(KO):
                dq().dma_start(
                    xT_sb[:, ko, 0:512], xT_d[ko * P : (ko + 1) * P, 0:512]
                )
            for ko in range(KO):
                dq().dma_start(wvT_sb[:, ko, :], wvT_d[ko * P : (ko + 1) * P, :])
            dq().dma_start(maskT_sb[:], maskT_d[:])
            for qc in (1, 2):
                for ko in range(KO):
                    dq().dma_start(
                        xT_sb[:, ko, qc * 512 : (qc + 1) * 512],
                        xT_d[ko * P : (ko + 1) * P, qc * 512 : (qc + 1) * 512],
                    )
            dq().dma_start(woT_sb[:], woT_d.rearrange("(ct p) e -> p ct e", p=P))
            for ko in range(KO):
                dq().dma_start(
                    xT_sb[:, ko, 3 * 512 : 4 * 512],
                    xT_d[ko * P : (ko + 1) * P, 3 * 512 : 4 * 512],
                )

            ones_f32 = sb_small.tile([P, DH], F32, tag="ones")
            nc.vector.memset(ones_f32[:], 1.0)
            nc.vector.tensor_copy(
                out=v_sb[:, :, :, DH],
                in_=ones_f32[:, 0 : 4 * NQ * HPC].rearrange(
                    "p (a b) -> p a b", a=4 * NQ
                ),
            )

            # ---------- filler units (PE work with no ACT dependency) ----------
            def qk_chain(qc, slot):
                ch = pp_ch.tile([P, 512], F32, tag="ch")
                for ko in range(KO):
                    nc.tensor.matmul(
                        ch[:],
                        wqkT_sb[:, ko, slot * P : (slot + 1) * P],
                        xT_sb[:, ko, qc * 512 : (qc + 1) * 512],
                        start=(ko == 0),
                        stop=(ko == KO - 1),
                        skip_group_check=True,
                    )
                nc.vector.tensor_copy(
                    out=qkT_sb[:, slot, qc * 512 : (qc + 1) * 512], in_=ch[:]
                )

            def v_chain(qc, j):
                sc = 4 * qc + j
                ch = pp_ch.tile([P, 512], F32, tag="ch")
                for ko in range(KO):
                    nc.tensor.matmul(
                        ch[:, 0:VC],
                        xT_sb[:, ko, sc * P : (sc + 1) * P],
                        wvT_sb[:, ko, :],
                        start=(ko == 0),
                        stop=(ko == KO - 1),
                        skip_group_check=True,
                    )
                nc.vector.tensor_copy(
                    out=v_sb[:, sc, :, 0:DH],
                    in_=ch[:, 0:VC].rearrange("p (h d) -> p h d", h=HPC),
                )

            def outproj_unit(sc, en, tail=False):
                ps_o = pp_ch.tile([P, 512], F32, tag="ch")
                for ct in range(2):
                    nc.tensor.matmul(
                        ps_o[:],
                        attn_sb[:, ct, sc * P : (sc + 1) * P],
                        woT_sb[:, ct, en * 512 : (en + 1) * 512],
                        start=(ct == 0),
                        stop=(ct == 1),
                        skip_group_check=True,
                    )
                o_sb = sb_out.tile([P, 512], F32, tag="osb")
                nc.vector.tensor_copy(out=o_sb[:], in_=ps_o[:])
                nc.sync.dma_start(
                    out_d[sc * P : (sc + 1) * P, en * 512 : (en + 1) * 512],
                    o_sb[:],
                )

            # ---------- attention for one (qc, head), weaving fillers ----------
            def attention_head(qc, h, fillers, post):
                hp = (h % 2) * DH
                mq = h // 2
                nkb = 4 * qc + 4
                avs = []
                pair_idx = 0
                for kb0 in range(0, nkb, 2):
                    ps2 = pp_s2.tile([P, 1024], F32, tag="s2")
                    exp2 = sb_exp.tile([P, 1024], CDT, tag="exp")
                    offs = []
                    for half in (0, 1):
                        kb = kb0 + half
                        m = kb - 4 * qc
                        off = max(0, m) * P
                        offs.append(off)
                        nc.tensor.matmul(
                            ps2[:, half * 512 + off : half * 512 + 512],
                            qkT_sb[hp : hp + DH, 2 + mq, kb * P : (kb + 1) * P],
                            qkT_sb[
                                hp : hp + DH, mq, qc * 512 + off : (qc + 1) * 512
                            ],
                            start=True,
                            stop=True,
                            skip_group_check=True,
                        )
                    if offs[0] == 0 and offs[1] == 0:
                        nc.scalar.activation(exp2[:], ps2[:], EXP, scale=0.125)
                    else:
                        for half, off in enumerate(offs):
                            lo = half * 512 + off
                            nc.scalar.activation(
                                exp2[:, lo : half * 512 + 512],
                                ps2[:, lo : half * 512 + 512],
                                EXP,
                                scale=0.125,
                            )
                    for half, off in enumerate(offs):
                        kb = kb0 + half
                        if kb - 4 * qc >= 0:
                            lo = half * 512 + off
                            nc.vector.tensor_mul(
                                out=exp2[:, lo : lo + P],
                                in0=exp2[:, lo : lo + P],
                                in1=maskT_sb[:],
                            )
                        avs.append((exp2, half * 512 + off, off, kb))
                    pair_idx += 1
                    if pair_idx % 2 == 0 and fillers:
                        fillers.popleft()()
                ps_av = pp_av.tile([DH + 1, 512], F32, tag="av")
                for j, (exp2, lo, off, kb) in enumerate(avs):
                    nc.tensor.matmul(
                        ps_av[:, off:512],
                        v_sb[:, kb, h, :],
                        exp2[:, lo : (lo - off) + 512],
                        start=(j == 0),
                        stop=(j == len(avs) - 1),
                        skip_group_check=True,
                    )
                sums_sb = sb_small.tile([1, 512], F32, tag="sums")
                nc.vector.tensor_copy(out=sums_sb[:], in_=ps_av[DH : DH + 1, :])
                recip_f = sb_small.tile([1, 512], F32, tag="recipf")
                nc.vector.reciprocal_approx_fast(out=recip_f[:], in_=sums_sb[:])
                bc_sb = sb_small.tile([DH, 512], F32, tag="bc")
                nc.gpsimd.partition_broadcast(bc_sb[:], recip_f[:])
                nc.vector.tensor_mul(
                    out=attn_sb[hp : hp + DH, mq, qc * 512 : (qc + 1) * 512],
                    in0=ps_av[0:DH, :],
                    in1=bc_sb[:],
                )
                for _ in range(post):
                    if fillers:
                        fillers.popleft()()

            # ---------- schedule ----------
            from collections import deque

            # bootstrap: projections for qc0
            for slot in range(4):
                qk_chain(0, slot)
            for j in range(4):
                v_chain(0, j)

            # window fillers: PE work without ACT deps, woven into attention
            win = {
                0: [lambda s=s: qk_chain(1, s) for s in range(4)]
                + [lambda j=j: v_chain(1, j) for j in range(4)],
                1: [lambda s=s: qk_chain(2, s) for s in range(4)]
                + [lambda j=j: v_chain(2, j) for j in range(4)]
                + [lambda sc=sc, en=en: outproj_unit(sc, en)
                   for sc in range(4) for en in range(2)],
                2: [lambda s=s: qk_chain(3, s) for s in range(4)]
                + [lambda sc=sc, en=en: outproj_unit(sc, en)
                   for sc in range(4, 8) for en in range(2)][:4],
                3: [lambda j=j: v_chain(3, j) for j in range(4)]
                + [lambda sc=sc, en=en: outproj_unit(sc, en)
                   for sc in range(4, 8) for en in range(2)][4:]
                + [lambda sc=sc, en=en: outproj_unit(sc, en)
                   for sc in range(8, 12) for en in range(2)],
            }
            post = {0: 1, 1: 2, 2: 1, 3: 1}
            for qc in range(NQ):
                fillers = deque(win[qc])
                for h in range(HPC):
                    attention_head(qc, h, fillers, post[qc])
                while fillers:
                    fillers.popleft()()
            # tail: outproj for q-chunks 12..15
            for sc in range(12, 16):
                for en in range(2):
                    outproj_unit(sc, en, tail=True)

    nc.compile()
    return nc


def _get_nc():
    if "nc" not in _cache:
        _cache["nc"] = _build()
    return _cache["nc"]


def _shard(x, mask, Wqkv, Wo):
    import ml_dtypes

    cdt = ml_dtypes.bfloat16
    in_maps = []
    maskT = np.ascontiguousarray((mask[0, 0, :P, :P].T >= 0).astype(cdt))
    for c in range(NCORES):
        b = c // 4
        g = c % 4
        heads = [4 * g + i for i in range(HPC)]
        q_rows = np.concatenate([np.arange(h * DH, (h + 1) * DH) for h in heads])
        k_rows = D + q_rows
        v_rows = 2 * D + q_rows
        qk_rows = np.concatenate([q_rows, k_rows])
        in_maps.append(
            {
                "xT": np.ascontiguousarray(x[b].T.astype(cdt)),
                "wqkT": np.ascontiguousarray(Wqkv[qk_rows, :].T.astype(cdt)),
                "wvT": np.ascontiguousarray(Wqkv[v_rows, :].T.astype(cdt)),
                "woT": np.ascontiguousarray(Wo[:, q_rows].T.astype(cdt)),
                "maskT": maskT,
            }
        )
    return in_maps


def kernel(x, mask, Wqkv, Wo, _trace=False):
    from concourse.bass_utils import run_bass_kernel_spmd

    x = np.asarray(x, dtype=np.float32)
    mask = np.asarray(mask, dtype=np.float32)
    Wqkv = np.asarray(Wqkv, dtype=np.float32)
    Wo = np.asarray(Wo, dtype=np.float32)

    nc = _get_nc()
    in_maps = _shard(x, mask, Wqkv, Wo)
    res = run_bass_kernel_spmd(nc, in_maps, core_ids=list(range(NCORES)), trace=_trace)
    _cache["last_result"] = res

    out = np.zeros((B, S, D), dtype=np.float32)
    for c in range(NCORES):
        out[c // 4] += np.asarray(res.results[c]["out"], dtype=np.float32)
    return out


# revision 17
# speedup vs baseline: 1.0118x; 1.0001x over previous
"""Causal self-attention Trainium2 kernel (8-core SPMD), v2.

Problem: x[2,2048,1024], causal mask, Wqkv[3072,1024], Wo[1024,1024], fp32.
  qkv = x @ Wqkv.T ; per-head causal softmax attention ; out = attn @ Wo.T

Sharding (data + tensor parallel over heads):
  core c -> batch b = c // 4, heads {4g..4g+3} with g = c % 4.
  Each core computes Q,K,V for its 4 heads, runs causal attention, applies the
  matching 256 columns of Wo, and writes a partial [2048,1024] output; the
  host sums the 4 partials per batch.

v2 changes vs v1 (trace-driven):
  - Projections emitted as compact 8-matmul chains into 1-bank PSUM tiles and
    woven between attention score-pairs as PE filler, so the PE never idles
    while ACT exp (the slowest per-block stage, ~1.1us per 1024 cols) runs.
  - Output projection for q-chunk qc emitted as filler during later windows
    instead of serially at the end (smaller tail).
  - DMA issued in first-needed order ([128,512] x pieces, qc-major) across 4
    engine queues, so the first projection chain starts ~6us earlier.
  - Normalization reciprocal reads the AV PSUM row directly (one less copy).
  - All-bf16 compute (fp8 rejected: DR matmuls gave no net win at equal
    column rate and the error budget was too tight).
"""

import numpy as np

S = 2048
D = 1024
DH = 64
B = 2
NCORES = 8
HPC = 4  # heads per core
QKC = 2 * HPC * DH  # 512 q+k projection columns per core
VC = HPC * DH  # 256 v columns per core
P = 128
KO = D // P  # 8 contraction tiles
NQ = S // 512  # 4 q-chunks of 512

_cache = {}


def _build():
    import concourse.bacc as bacc
    import concourse.mybir as mybir
    import concourse.tile as tile

    F32 = mybir.dt.float32
    CDT = mybir.dt.bfloat16
    EXP = mybir.ActivationFunctionType.Exp

    nc = bacc.Bacc()
    xT_d = nc.dram_tensor("xT", [D, S], CDT, kind="ExternalInput")
    wqkT_d = nc.dram_tensor("wqkT", [D, QKC], CDT, kind="ExternalInput")
    wvT_d = nc.dram_tensor("wvT", [D, VC], CDT, kind="ExternalInput")
    woT_d = nc.dram_tensor("woT", [VC, D], CDT, kind="ExternalInput")
    maskT_d = nc.dram_tensor("maskT", [P, P], CDT, kind="ExternalInput")
    out_d = nc.dram_tensor("out", [S, D], F32, kind="ExternalOutput")

    with tile.TileContext(nc) as tc:
        with (
            tc.tile_pool(name="persist", bufs=1) as persist,
            tc.tile_pool(name="sb_small", bufs=4) as sb_small,
            tc.tile_pool(name="sb_exp", bufs=6) as sb_exp,
            tc.tile_pool(name="sb_out", bufs=4) as sb_out,
            tc.tile_pool(name="pp_s2", bufs=2, space="PSUM") as pp_s2,
            tc.tile_pool(name="pp_ch", bufs=2, space="PSUM") as pp_ch,
            tc.tile_pool(name="pp_av", bufs=2, space="PSUM") as pp_av,
        ):
            xT_sb = persist.tile([P, KO, S], CDT, tag="xT")
            wqkT_sb = persist.tile([P, KO, QKC], CDT, tag="wqkT")
            wvT_sb = persist.tile([P, KO, VC], CDT, tag="wvT")
            woT_sb = persist.tile([P, 2, D], CDT, tag="woT")
            maskT_sb = persist.tile([P, P], CDT, tag="maskT")
            qkT_sb = persist.tile([P, 4, S], CDT, tag="qkT")
            v_sb = persist.tile([P, 4 * NQ, HPC, DH + 1], CDT, tag="v")
            attn_sb = persist.tile([P, 2, S], CDT, tag="attn")

            # --- input DMAs: [128,512] pieces in first-needed order, 3 queues ---
            qs = [nc.sync, nc.gpsimd, nc.scalar]
            di = 0

            def dq():
                nonlocal di
                e = qs[di % 3]
                di += 1
                return e

            for ko in range(KO):
                dq().dma_start(wqkT_sb[:, ko, :], wqkT_d[ko * P : (ko + 1) * P, :])
            for ko in range(KO):
                dq().dma_start(
                    xT_sb[:, ko, 0:512], xT_d[ko * P : (ko + 1) * P, 0:512]
                )
            for ko in range(KO):
                dq().dma_start(wvT_sb[:, ko, :], wvT_d[ko * P : (ko + 1) * P, :])
            dq().dma_start(maskT_sb[:], maskT_d[:])
            for qc in (1, 2):
                for ko in range(KO):
                    dq().dma_start(
                        xT_sb[:, ko, qc * 512 : (qc + 1) * 512],
                        xT_d[ko * P : (ko + 1) * P, qc * 512 : (qc + 1) * 512],
                    )
            dq().dma_start(woT_sb[:], woT_d.rearrange("(ct p) e -> p ct e", p=P))
            for ko in range(KO):
                dq().dma_start(
                    xT_sb[:, ko, 3 * 512 : 4 * 512],
                    xT_d[ko * P : (ko + 1) * P, 3 * 512 : 4 * 512],
                )

            ones_f32 = sb_small.tile([P, DH], F32, tag="ones")
            nc.vector.memset(ones_f32[:], 1.0)
            nc.vector.tensor_copy(
                out=v_sb[:, :, :, DH],
                in_=ones_f32[:, 0 : 4 * NQ * HPC].rearrange(
                    "p (a b) -> p a b", a=4 * NQ
                ),
            )

            # ---------- filler units (PE work with no ACT dependency) ----------
            def qk_chain(qc, slot):
                ch = pp_ch.tile([P, 512], F32, tag="ch")
                for ko in range(KO):
                    nc.tensor.matmul(
                        ch[:],
                        wqkT_sb[:, ko, slot * P : (slot + 1) * P],
                        xT_sb[:, ko, qc * 512 : (qc + 1) * 512],
                        start=(ko == 0),
                        stop=(ko == KO - 1),
                        skip_group_check=True,
                    )
                nc.vector.tensor_copy(
                    out=qkT_sb[:, slot, qc * 512 : (qc + 1) * 512], in_=ch[:]
                )

            def v_chain(qc, j):
                sc = 4 * qc + j
                ch = pp_ch.tile([P, 512], F32, tag="ch")
                for ko in range(KO):
                    nc.tensor.matmul(
                        ch[:, 0:VC],
                        xT_sb[:, ko, sc * P : (sc + 1) * P],
                        wvT_sb[:, ko, :],
                        start=(ko == 0),
                        stop=(ko == KO - 1),
                        skip_group_check=True,
                    )
                nc.vector.tensor_copy(
                    out=v_sb[:, sc, :, 0:DH],
                    in_=ch[:, 0:VC].rearrange("p (h d) -> p h d", h=HPC),
                )

            def outproj_unit(sc, en, tail=False):
                ps_o = pp_ch.tile([P, 512], F32, tag="ch")
                for ct in range(2):
                    nc.tensor.matmul(
                        ps_o[:],
                        attn_sb[:, ct, sc * P : (sc + 1) * P],
                        woT_sb[:, ct, en * 512 : (en + 1) * 512],
                        start=(ct == 0),
                        stop=(ct == 1),
                        skip_group_check=True,
                    )
                o_sb = sb_out.tile([P, 512], F32, tag="osb")
                nc.vector.tensor_copy(out=o_sb[:], in_=ps_o[:])
                nc.sync.dma_start(
                    out_d[sc * P : (sc + 1) * P, en * 512 : (en + 1) * 512],
                    o_sb[:],
                )

            # ---------- attention for one (qc, head), weaving fillers ----------
            def attention_head(qc, h, fillers, post=99):
                hp = (h % 2) * DH
                mq = h // 2
                nkb = 4 * qc + 4
                avs = []
                pair_idx = 0
                for kb0 in range(0, nkb, 2):
                    ps2 = pp_s2.tile([P, 1024], F32, tag="s2")
                    exp2 = sb_exp.tile([P, 1024], CDT, tag="exp")
                    offs = []
                    for half in (0, 1):
                        kb = kb0 + half
                        m = kb - 4 * qc
                        off = max(0, m) * P
                        offs.append(off)
                        nc.tensor.matmul(
                            ps2[:, half * 512 + off : half * 512 + 512],
                            qkT_sb[hp : hp + DH, 2 + mq, kb * P : (kb + 1) * P],
                            qkT_sb[
                                hp : hp + DH, mq, qc * 512 + off : (qc + 1) * 512
                            ],
                            start=True,
                            stop=True,
                            skip_group_check=True,
                        )
                    if offs[0] == 0 and offs[1] == 0:
                        nc.scalar.activation(exp2[:], ps2[:], EXP, scale=0.125)
                    else:
                        for half, off in enumerate(offs):
                            lo = half * 512 + off
                            nc.scalar.activation(
                                exp2[:, lo : half * 512 + 512],
                                ps2[:, lo : half * 512 + 512],
                                EXP,
                                scale=0.125,
                            )
                    for half, off in enumerate(offs):
                        kb = kb0 + half
                        if kb - 4 * qc >= 0:
                            lo = half * 512 + off
                            nc.vector.tensor_mul(
                                out=exp2[:, lo : lo + P],
                                in0=exp2[:, lo : lo + P],
                                in1=maskT_sb[:],
                            )
                        avs.append((exp2, half * 512 + off, off, kb))
                    pair_idx += 1
                    if pair_idx % 2 == 0 and fillers:
                        fillers.popleft()()
                ps_av = pp_av.tile([DH + 1, 512], F32, tag="av")
                for j, (exp2, lo, off, kb) in enumerate(avs):
                    nc.tensor.matmul(
                        ps_av[:, off:512],
                        v_sb[:, kb, h, :],
                        exp2[:, lo : (lo - off) + 512],
                        start=(j == 0),
                        stop=(j == len(avs) - 1),
                        skip_group_check=True,
                    )
                sums_sb = sb_small.tile([1, 512], F32, tag="sums")
                nc.vector.tensor_copy(out=sums_sb[:], in_=ps_av[DH : DH + 1, :])
                recip_f = sb_small.tile([1, 512], F32, tag="recipf")
                nc.vector.reciprocal_approx_fast(out=recip_f[:], in_=sums_sb[:])
                bc_sb = sb_small.tile([DH, 512], F32, tag="bc")
                nc.gpsimd.partition_broadcast(bc_sb[:], recip_f[:])
                nc.vector.tensor_mul(
                    out=attn_sb[hp : hp + DH, mq, qc * 512 : (qc + 1) * 512],
                    in0=ps_av[0:DH, :],
                    in1=bc_sb[:],
                )
                for _ in range(post):
                    if fillers:
                        fillers.popleft()()

            # ---------- schedule ----------
            from collections import deque

            # bootstrap: projections for qc0
            for slot in range(4):
                qk_chain(0, slot)
            for j in range(4):
                v_chain(0, j)

            def qk_u(qc, s):
                return lambda: qk_chain(qc, s)

            def v_u(qc, j):
                return lambda: v_chain(qc, j)

            def o_u(sc, en):
                return lambda: outproj_unit(sc, en)

            def outs(*scs):
                return [o_u(sc, en) for sc in scs for en in range(2)]

            # (qc, h, fillers): heads of qc2/qc3 interleaved so the last
            # head's exps (which gate the tail) start earlier.
            sched = [
                (0, 0, [qk_u(1, 0), qk_u(1, 1)]),
                (0, 1, [qk_u(1, 2), qk_u(1, 3)]),
                (0, 2, [v_u(1, 0), v_u(1, 1)]),
                (0, 3, [v_u(1, 2), v_u(1, 3)]),
                (1, 0, [qk_u(2, s) for s in range(4)]),
                (1, 1, [v_u(2, j) for j in range(4)]),
                (1, 2, outs(0, 1)),
                (1, 3, outs(2, 3)),
                (2, 0, [qk_u(3, s) for s in range(4)]),
                (2, 1, [v_u(3, j) for j in range(4)]),
                (3, 0, outs(4, 5)),
                (2, 2, outs(6, 7)),
                (3, 1, []),
                (2, 3, []),
                (3, 2, outs(8, 9)),
                (3, 3, outs(10, 11)),
            ]
            for qc, h, fl in sched:
                attention_head(qc, h, deque(fl))
            # tail: outproj for q-chunks 12..15
            for sc in range(12, 16):
                for en in range(2):
                    outproj_unit(sc, en, tail=True)

    nc.compile()
    return nc


def _get_nc():
    if "nc" not in _cache:
        _cache["nc"] = _build()
    return _cache["nc"]


def _shard(x, mask, Wqkv, Wo):
    import ml_dtypes

    cdt = ml_dtypes.bfloat16
    in_maps = []
    maskT = np.ascontiguousarray((mask[0, 0, :P, :P].T >= 0).astype(cdt))
    for c in range(NCORES):
        b = c // 4
        g = c % 4
        heads = [4 * g + i for i in range(HPC)]
        q_rows = np.concatenate([np.arange(h * DH, (h + 1) * DH) for h in heads])
        k_rows = D + q_rows
        v_rows = 2 * D + q_rows
        qk_rows = np.concatenate([q_rows, k_rows])
        in_maps.append(
            {
                "xT": np.ascontiguousarray(x[b].T.astype(cdt)),
                "wqkT": np.ascontiguousarray(Wqkv[qk_rows, :].T.astype(cdt)),
                "wvT": np.ascontiguousarray(Wqkv[v_rows, :].T.astype(cdt)),
                "woT": np.ascontiguousarray(Wo[:, q_rows].T.astype(cdt)),
                "maskT": maskT,
            }
        )
    return in_maps


def kernel(x, mask, Wqkv, Wo, _trace=False):
    from concourse.bass_utils import run_bass_kernel_spmd

    x = np.asarray(x, dtype=np.float32)
    mask = np.asarray(mask, dtype=np.float32)
    Wqkv = np.asarray(Wqkv, dtype=np.float32)
    Wo = np.asarray(Wo, dtype=np.float32)

    nc = _get_nc()
    in_maps = _shard(x, mask, Wqkv, Wo)
    res = run_bass_kernel_spmd(nc, in_maps, core_ids=list(range(NCORES)), trace=_trace)
    _cache["last_result"] = res

    out = np.zeros((B, S, D), dtype=np.float32)
    for c in range(NCORES):
        out[c // 4] += np.asarray(res.results[c]["out"], dtype=np.float32)
    return out


# revision 18
# speedup vs baseline: 1.0145x; 1.0027x over previous
"""Causal self-attention Trainium2 kernel (8-core SPMD), v2.

Problem: x[2,2048,1024], causal mask, Wqkv[3072,1024], Wo[1024,1024], fp32.
  qkv = x @ Wqkv.T ; per-head causal softmax attention ; out = attn @ Wo.T

Sharding (data + tensor parallel over heads):
  core c -> batch b = c // 4, heads {4g..4g+3} with g = c % 4.
  Each core computes Q,K,V for its 4 heads, runs causal attention, applies the
  matching 256 columns of Wo, and writes a partial [2048,1024] output; the
  host sums the 4 partials per batch.

v2 changes vs v1 (trace-driven):
  - Projections emitted as compact 8-matmul chains into 1-bank PSUM tiles and
    woven between attention score-pairs as PE filler, so the PE never idles
    while ACT exp (the slowest per-block stage, ~1.1us per 1024 cols) runs.
  - Output projection for q-chunk qc emitted as filler during later windows
    instead of serially at the end (smaller tail).
  - DMA issued in first-needed order ([128,512] x pieces, qc-major) across 4
    engine queues, so the first projection chain starts ~6us earlier.
  - Normalization reciprocal reads the AV PSUM row directly (one less copy).
  - All-bf16 compute (fp8 rejected: DR matmuls gave no net win at equal
    column rate and the error budget was too tight).
"""

import numpy as np

S = 2048
D = 1024
DH = 64
B = 2
NCORES = 8
HPC = 4  # heads per core
QKC = 2 * HPC * DH  # 512 q+k projection columns per core
VC = HPC * DH  # 256 v columns per core
P = 128
KO = D // P  # 8 contraction tiles
NQ = S // 512  # 4 q-chunks of 512

_cache = {}


def _build():
    import concourse.bacc as bacc
    import concourse.mybir as mybir
    import concourse.tile as tile

    F32 = mybir.dt.float32
    CDT = mybir.dt.bfloat16
    EXP = mybir.ActivationFunctionType.Exp

    nc = bacc.Bacc()
    FP8 = mybir.dt.float8e4
    DR = mybir.MatmulPerfMode.DoubleRow
    xT_d = nc.dram_tensor("xT", [D, S], CDT, kind="ExternalInput")
    xT8_d = nc.dram_tensor("xT8", [D, S], FP8, kind="ExternalInput")
    wqkT_d = nc.dram_tensor("wqkT", [D, QKC], FP8, kind="ExternalInput")
    wvT_d = nc.dram_tensor("wvT", [D, VC], CDT, kind="ExternalInput")
    woT_d = nc.dram_tensor("woT", [VC, D], CDT, kind="ExternalInput")
    maskT_d = nc.dram_tensor("maskT", [P, P], CDT, kind="ExternalInput")
    out_d = nc.dram_tensor("out", [S, D], F32, kind="ExternalOutput")

    with tile.TileContext(nc) as tc:
        with (
            tc.tile_pool(name="persist", bufs=1) as persist,
            tc.tile_pool(name="sb_small", bufs=4) as sb_small,
            tc.tile_pool(name="sb_exp", bufs=6) as sb_exp,
            tc.tile_pool(name="sb_out", bufs=4) as sb_out,
            tc.tile_pool(name="pp_s2", bufs=2, space="PSUM") as pp_s2,
            tc.tile_pool(name="pp_ch", bufs=2, space="PSUM") as pp_ch,
            tc.tile_pool(name="pp_av", bufs=2, space="PSUM") as pp_av,
        ):
            xT_sb = persist.tile([P, KO, S], CDT, tag="xT")
            xT8_sb = persist.tile([P, KO, S], FP8, tag="xT8")
            wqkT_sb = persist.tile([P, KO, QKC], FP8, tag="wqkT")
            wvT_sb = persist.tile([P, KO, VC], CDT, tag="wvT")
            woT_sb = persist.tile([P, 2, D], CDT, tag="woT")
            maskT_sb = persist.tile([P, P], CDT, tag="maskT")
            qkT_sb = persist.tile([P, 4, S], CDT, tag="qkT")
            v_sb = persist.tile([P, 4 * NQ, HPC, DH + 1], CDT, tag="v")
            attn_sb = persist.tile([P, 2, S], CDT, tag="attn")

            # --- input DMAs: [128,512] pieces in first-needed order, 3 queues ---
            qs = [nc.sync, nc.gpsimd, nc.scalar]
            di = 0

            def dq():
                nonlocal di
                e = qs[di % 3]
                di += 1
                return e

            for ko in range(KO):
                dq().dma_start(wqkT_sb[:, ko, :], wqkT_d[ko * P : (ko + 1) * P, :])
            for ko in range(KO):
                dq().dma_start(
                    xT8_sb[:, ko, 0:512], xT8_d[ko * P : (ko + 1) * P, 0:512]
                )
            for ko in range(KO):
                dq().dma_start(
                    xT_sb[:, ko, 0:512], xT_d[ko * P : (ko + 1) * P, 0:512]
                )
            for ko in range(KO):
                dq().dma_start(wvT_sb[:, ko, :], wvT_d[ko * P : (ko + 1) * P, :])
            dq().dma_start(maskT_sb[:], maskT_d[:])
            for qc in (1, 2, 3):
                for ko in range(KO):
                    dq().dma_start(
                        xT8_sb[:, ko, qc * 512 : (qc + 1) * 512],
                        xT8_d[ko * P : (ko + 1) * P, qc * 512 : (qc + 1) * 512],
                    )
                if qc == 2:
                    dq().dma_start(
                        woT_sb[:], woT_d.rearrange("(ct p) e -> p ct e", p=P)
                    )
                for ko in range(KO):
                    dq().dma_start(
                        xT_sb[:, ko, qc * 512 : (qc + 1) * 512],
                        xT_d[ko * P : (ko + 1) * P, qc * 512 : (qc + 1) * 512],
                    )

            ones_f32 = sb_small.tile([P, DH], F32, tag="ones")
            nc.vector.memset(ones_f32[:], 1.0)
            nc.vector.tensor_copy(
                out=v_sb[:, :, :, DH],
                in_=ones_f32[:, 0 : 4 * NQ * HPC].rearrange(
                    "p (a b) -> p a b", a=4 * NQ
                ),
            )

            # ---------- filler units (PE work with no ACT dependency) ----------
            def qk_chain(qc, slot):
                ch = pp_ch.tile([P, 512], F32, tag="ch")
                for kp in range(KO // 2):
                    nc.tensor.matmul(
                        ch[:],
                        wqkT_sb[:, 2 * kp : 2 * kp + 2, slot * P : (slot + 1) * P],
                        xT8_sb[:, 2 * kp : 2 * kp + 2, qc * 512 : (qc + 1) * 512],
                        start=(kp == 0),
                        stop=(kp == KO // 2 - 1),
                        perf_mode=DR,
                        skip_group_check=True,
                    )
                nc.vector.tensor_copy(
                    out=qkT_sb[:, slot, qc * 512 : (qc + 1) * 512], in_=ch[:]
                )

            def v_chain(qc, j):
                sc = 4 * qc + j
                ch = pp_ch.tile([P, 512], F32, tag="ch")
                for ko in range(KO):
                    nc.tensor.matmul(
                        ch[:, 0:VC],
                        xT_sb[:, ko, sc * P : (sc + 1) * P],
                        wvT_sb[:, ko, :],
                        start=(ko == 0),
                        stop=(ko == KO - 1),
                        skip_group_check=True,
                    )
                nc.vector.tensor_copy(
                    out=v_sb[:, sc, :, 0:DH],
                    in_=ch[:, 0:VC].rearrange("p (h d) -> p h d", h=HPC),
                )

            def outproj_unit(sc, en, tail=False):
                ps_o = pp_ch.tile([P, 512], F32, tag="ch")
                for ct in range(2):
                    nc.tensor.matmul(
                        ps_o[:],
                        attn_sb[:, ct, sc * P : (sc + 1) * P],
                        woT_sb[:, ct, en * 512 : (en + 1) * 512],
                        start=(ct == 0),
                        stop=(ct == 1),
                        skip_group_check=True,
                    )
                o_sb = sb_out.tile([P, 512], F32, tag="osb")
                nc.vector.tensor_copy(out=o_sb[:], in_=ps_o[:])
                nc.sync.dma_start(
                    out_d[sc * P : (sc + 1) * P, en * 512 : (en + 1) * 512],
                    o_sb[:],
                )

            # ---------- attention for one (qc, head), weaving fillers ----------
            def attention_head(qc, h, fillers, post=99):
                hp = (h % 2) * DH
                mq = h // 2
                nkb = 4 * qc + 4
                avs = []
                pair_idx = 0
                for kb0 in range(0, nkb, 2):
                    ps2 = pp_s2.tile([P, 1024], F32, tag="s2")
                    exp2 = sb_exp.tile([P, 1024], CDT, tag="exp")
                    offs = []
                    for half in (0, 1):
                        kb = kb0 + half
                        m = kb - 4 * qc
                        off = max(0, m) * P
                        offs.append(off)
                        nc.tensor.matmul(
                            ps2[:, half * 512 + off : half * 512 + 512],
                            qkT_sb[hp : hp + DH, 2 + mq, kb * P : (kb + 1) * P],
                            qkT_sb[
                                hp : hp + DH, mq, qc * 512 + off : (qc + 1) * 512
                            ],
                            start=True,
                            stop=True,
                            skip_group_check=True,
                        )
                    if offs[0] == 0 and offs[1] == 0:
                        nc.scalar.activation(exp2[:], ps2[:], EXP, scale=0.125)
                    else:
                        for half, off in enumerate(offs):
                            lo = half * 512 + off
                            nc.scalar.activation(
                                exp2[:, lo : half * 512 + 512],
                                ps2[:, lo : half * 512 + 512],
                                EXP,
                                scale=0.125,
                            )
                    for half, off in enumerate(offs):
                        kb = kb0 + half
                        if kb - 4 * qc >= 0:
                            lo = half * 512 + off
                            nc.vector.tensor_mul(
                                out=exp2[:, lo : lo + P],
                                in0=exp2[:, lo : lo + P],
                                in1=maskT_sb[:],
                            )
                        avs.append((exp2, half * 512 + off, off, kb))
                    pair_idx += 1
                    if pair_idx % 2 == 0 and fillers:
                        fillers.popleft()()
                ps_av = pp_av.tile([DH + 1, 512], F32, tag="av")
                for j, (exp2, lo, off, kb) in enumerate(avs):
                    nc.tensor.matmul(
                        ps_av[:, off:512],
                        v_sb[:, kb, h, :],
                        exp2[:, lo : (lo - off) + 512],
                        start=(j == 0),
                        stop=(j == len(avs) - 1),
                        skip_group_check=True,
                    )
                sums_sb = sb_small.tile([1, 512], F32, tag="sums")
                nc.vector.tensor_copy(out=sums_sb[:], in_=ps_av[DH : DH + 1, :])
                recip_f = sb_small.tile([1, 512], F32, tag="recipf")
                nc.vector.reciprocal_approx_fast(out=recip_f[:], in_=sums_sb[:])
                bc_sb = sb_small.tile([DH, 512], F32, tag="bc")
                nc.gpsimd.partition_broadcast(bc_sb[:], recip_f[:])
                nc.vector.tensor_mul(
                    out=attn_sb[hp : hp + DH, mq, qc * 512 : (qc + 1) * 512],
                    in0=ps_av[0:DH, :],
                    in1=bc_sb[:],
                )
                for _ in range(post):
                    if fillers:
                        fillers.popleft()()

            # ---------- schedule ----------
            from collections import deque

            # bootstrap: projections for qc0
            for slot in range(4):
                qk_chain(0, slot)
            for j in range(4):
                v_chain(0, j)

            def qk_u(qc, s):
                return lambda: qk_chain(qc, s)

            def v_u(qc, j):
                return lambda: v_chain(qc, j)

            def o_u(sc, en):
                return lambda: outproj_unit(sc, en)

            def outs(*scs):
                return [o_u(sc, en) for sc in scs for en in range(2)]

            # (qc, h, fillers): heads of qc2/qc3 interleaved so the last
            # head's exps (which gate the tail) start earlier.
            sched = [
                (0, 0, [qk_u(1, 0), qk_u(1, 1)]),
                (0, 1, [qk_u(1, 2), qk_u(1, 3)]),
                (0, 2, [v_u(1, 0), v_u(1, 1)]),
                (0, 3, [v_u(1, 2), v_u(1, 3)]),
                (1, 0, [qk_u(2, s) for s in range(4)]),
                (1, 1, [v_u(2, j) for j in range(4)]),
                (1, 2, outs(0, 1)),
                (1, 3, outs(2, 3)),
                (2, 0, [qk_u(3, s) for s in range(4)]),
                (2, 1, [v_u(3, j) for j in range(4)]),
                (3, 0, outs(4, 5)),
                (2, 2, outs(6, 7)),
                (3, 1, []),
                (2, 3, []),
                (3, 2, outs(8, 9)),
                (3, 3, outs(10, 11)),
            ]
            for qc, h, fl in sched:
                attention_head(qc, h, deque(fl))
            # tail: outproj for q-chunks 12..15
            for sc in range(12, 16):
                for en in range(2):
                    outproj_unit(sc, en, tail=True)

    nc.compile()
    return nc


def _get_nc():
    if "nc" not in _cache:
        _cache["nc"] = _build()
    return _cache["nc"]


def _shard(x, mask, Wqkv, Wo):
    import ml_dtypes

    cdt = ml_dtypes.bfloat16
    in_maps = []
    maskT = np.ascontiguousarray((mask[0, 0, :P, :P].T >= 0).astype(cdt))
    for c in range(NCORES):
        b = c // 4
        g = c % 4
        heads = [4 * g + i for i in range(HPC)]
        q_rows = np.concatenate([np.arange(h * DH, (h + 1) * DH) for h in heads])
        k_rows = D + q_rows
        v_rows = 2 * D + q_rows
        qk_rows = np.concatenate([q_rows, k_rows])
        in_maps.append(
            {
                "xT": np.ascontiguousarray(x[b].T.astype(cdt)),
                "xT8": np.ascontiguousarray(x[b].T.astype(ml_dtypes.float8_e4m3)),
                "wqkT": np.ascontiguousarray(
                    Wqkv[qk_rows, :].T.astype(ml_dtypes.float8_e4m3)
                ),
                "wvT": np.ascontiguousarray(Wqkv[v_rows, :].T.astype(cdt)),
                "woT": np.ascontiguousarray(Wo[:, q_rows].T.astype(cdt)),
                "maskT": maskT,
            }
        )
    return in_maps


def kernel(x, mask, Wqkv, Wo, _trace=False):
    from concourse.bass_utils import run_bass_kernel_spmd

    x = np.asarray(x, dtype=np.float32)
    mask = np.asarray(mask, dtype=np.float32)
    Wqkv = np.asarray(Wqkv, dtype=np.float32)
    Wo = np.asarray(Wo, dtype=np.float32)

    nc = _get_nc()
    in_maps = _shard(x, mask, Wqkv, Wo)
    res = run_bass_kernel_spmd(nc, in_maps, core_ids=list(range(NCORES)), trace=_trace)
    _cache["last_result"] = res

    out = np.zeros((B, S, D), dtype=np.float32)
    for c in range(NCORES):
        out[c // 4] += np.asarray(res.results[c]["out"], dtype=np.float32)
    return out


# revision 19
# speedup vs baseline: 1.0290x; 1.0143x over previous
"""Causal self-attention Trainium2 kernel (8-core SPMD), v2.

Problem: x[2,2048,1024], causal mask, Wqkv[3072,1024], Wo[1024,1024], fp32.
  qkv = x @ Wqkv.T ; per-head causal softmax attention ; out = attn @ Wo.T

Sharding (data + tensor parallel over heads):
  core c -> batch b = c // 4, heads {4g..4g+3} with g = c % 4.
  Each core computes Q,K,V for its 4 heads, runs causal attention, applies the
  matching 256 columns of Wo, and writes a partial [2048,1024] output; the
  host sums the 4 partials per batch.

v2 changes vs v1 (trace-driven):
  - Projections emitted as compact 8-matmul chains into 1-bank PSUM tiles and
    woven between attention score-pairs as PE filler, so the PE never idles
    while ACT exp (the slowest per-block stage, ~1.1us per 1024 cols) runs.
  - Output projection for q-chunk qc emitted as filler during later windows
    instead of serially at the end (smaller tail).
  - DMA issued in first-needed order ([128,512] x pieces, qc-major) across 4
    engine queues, so the first projection chain starts ~6us earlier.
  - Normalization reciprocal reads the AV PSUM row directly (one less copy).
  - All-bf16 compute (fp8 rejected: DR matmuls gave no net win at equal
    column rate and the error budget was too tight).
"""

import numpy as np

S = 2048
D = 1024
DH = 64
B = 2
NCORES = 8
HPC = 4  # heads per core
QKC = 2 * HPC * DH  # 512 q+k projection columns per core
VC = HPC * DH  # 256 v columns per core
P = 128
KO = D // P  # 8 contraction tiles
NQ = S // 512  # 4 q-chunks of 512

_cache = {}


def _build():
    import concourse.bacc as bacc
    import concourse.mybir as mybir
    import concourse.tile as tile

    F32 = mybir.dt.float32
    CDT = mybir.dt.bfloat16
    EXP = mybir.ActivationFunctionType.Exp

    nc = bacc.Bacc()
    FP8 = mybir.dt.float8e4
    DR = mybir.MatmulPerfMode.DoubleRow
    xT_d = nc.dram_tensor("xT", [D, S], CDT, kind="ExternalInput")
    xT8_d = nc.dram_tensor("xT8", [D, S], FP8, kind="ExternalInput")
    wqkT_d = nc.dram_tensor("wqkT", [D, QKC], FP8, kind="ExternalInput")
    wvT_d = nc.dram_tensor("wvT", [D, VC], CDT, kind="ExternalInput")
    woT_d = nc.dram_tensor("woT", [VC, D], CDT, kind="ExternalInput")
    maskT_d = nc.dram_tensor("maskT", [P, P], CDT, kind="ExternalInput")
    out_d = nc.dram_tensor("out", [S, D], F32, kind="ExternalOutput")

    with tile.TileContext(nc) as tc:
        with (
            tc.tile_pool(name="persist", bufs=1) as persist,
            tc.tile_pool(name="sb_small", bufs=4) as sb_small,
            tc.tile_pool(name="sb_exp", bufs=6) as sb_exp,
            tc.tile_pool(name="sb_out", bufs=4) as sb_out,
            tc.tile_pool(name="pp_s2", bufs=2, space="PSUM") as pp_s2,
            tc.tile_pool(name="pp_ch", bufs=2, space="PSUM") as pp_ch,
            tc.tile_pool(name="pp_av", bufs=2, space="PSUM") as pp_av,
        ):
            xT_sb = persist.tile([P, KO, S], CDT, tag="xT")
            xT8_sb = persist.tile([P, KO, S], FP8, tag="xT8")
            wqkT_sb = persist.tile([P, KO, QKC], FP8, tag="wqkT")
            wvT_sb = persist.tile([P, KO, VC], CDT, tag="wvT")
            woT_sb = persist.tile([P, 2, D], CDT, tag="woT")
            maskT_sb = persist.tile([P, P], CDT, tag="maskT")
            qkT_sb = persist.tile([P, 4, S], CDT, tag="qkT")
            v_sb = persist.tile([P, 4 * NQ, HPC, DH + 1], CDT, tag="v")
            attn_sb = persist.tile([P, 2, S], CDT, tag="attn")

            # --- input DMAs: [128,512] pieces in first-needed order, 3 queues ---
            qs = [nc.sync, nc.gpsimd, nc.scalar]
            di = 0

            def dq():
                nonlocal di
                e = qs[di % 3]
                di += 1
                return e

            for ko in range(KO):
                dq().dma_start(wqkT_sb[:, ko, :], wqkT_d[ko * P : (ko + 1) * P, :])
            for ko in range(KO):
                dq().dma_start(
                    xT8_sb[:, ko, 0:512], xT8_d[ko * P : (ko + 1) * P, 0:512]
                )
            for ko in range(KO):
                dq().dma_start(
                    xT_sb[:, ko, 0:512], xT_d[ko * P : (ko + 1) * P, 0:512]
                )
            for ko in range(KO):
                dq().dma_start(wvT_sb[:, ko, :], wvT_d[ko * P : (ko + 1) * P, :])
            dq().dma_start(maskT_sb[:], maskT_d[:])
            for qc in (1, 2, 3):
                for ko in range(KO):
                    dq().dma_start(
                        xT8_sb[:, ko, qc * 512 : (qc + 1) * 512],
                        xT8_d[ko * P : (ko + 1) * P, qc * 512 : (qc + 1) * 512],
                    )
                if qc == 2:
                    dq().dma_start(
                        woT_sb[:], woT_d.rearrange("(ct p) e -> p ct e", p=P)
                    )
                for ko in range(KO):
                    dq().dma_start(
                        xT_sb[:, ko, qc * 512 : (qc + 1) * 512],
                        xT_d[ko * P : (ko + 1) * P, qc * 512 : (qc + 1) * 512],
                    )

            ones_f32 = sb_small.tile([P, DH], F32, tag="ones")
            nc.vector.memset(ones_f32[:], 1.0)
            nc.vector.tensor_copy(
                out=v_sb[:, :, :, DH],
                in_=ones_f32[:, 0 : 4 * NQ * HPC].rearrange(
                    "p (a b) -> p a b", a=4 * NQ
                ),
            )

            # ---------- filler units (PE work with no ACT dependency) ----------
            def qk_chain(qc, slot):
                ch = pp_ch.tile([P, 512], F32, tag="ch")
                for kp in range(KO // 2):
                    nc.tensor.matmul(
                        ch[:],
                        wqkT_sb[:, 2 * kp : 2 * kp + 2, slot * P : (slot + 1) * P],
                        xT8_sb[:, 2 * kp : 2 * kp + 2, qc * 512 : (qc + 1) * 512],
                        start=(kp == 0),
                        stop=(kp == KO // 2 - 1),
                        perf_mode=DR,
                        skip_group_check=True,
                    )
                nc.vector.tensor_copy(
                    out=qkT_sb[:, slot, qc * 512 : (qc + 1) * 512], in_=ch[:]
                )

            def v_chain(qc, j):
                sc = 4 * qc + j
                ch = pp_ch.tile([P, 512], F32, tag="ch")
                for ko in range(KO):
                    nc.tensor.matmul(
                        ch[:, 0:VC],
                        xT_sb[:, ko, sc * P : (sc + 1) * P],
                        wvT_sb[:, ko, :],
                        start=(ko == 0),
                        stop=(ko == KO - 1),
                        skip_group_check=True,
                    )
                nc.vector.tensor_copy(
                    out=v_sb[:, sc, :, 0:DH],
                    in_=ch[:, 0:VC].rearrange("p (h d) -> p h d", h=HPC),
                )

            def outproj_unit(sc, en, tail=False):
                ps_o = pp_ch.tile([P, 512], F32, tag="ch")
                for ct in range(2):
                    nc.tensor.matmul(
                        ps_o[:],
                        attn_sb[:, ct, sc * P : (sc + 1) * P],
                        woT_sb[:, ct, en * 512 : (en + 1) * 512],
                        start=(ct == 0),
                        stop=(ct == 1),
                        skip_group_check=True,
                    )
                o_sb = sb_out.tile([P, 512], F32, tag="osb")
                nc.vector.tensor_copy(out=o_sb[:], in_=ps_o[:])
                nc.sync.dma_start(
                    out_d[sc * P : (sc + 1) * P, en * 512 : (en + 1) * 512],
                    o_sb[:],
                )

            # ---------- attention for one (qc, head), weaving fillers ----------
            def attention_head(qc, h, fillers, post=99):
                hp = (h % 2) * DH
                mq = h // 2
                nkb = 4 * qc + 4
                avs = []
                pair_idx = 0
                for kb0 in range(0, nkb, 2):
                    ps2 = pp_s2.tile([P, 1024], F32, tag="s2")
                    exp2 = sb_exp.tile([P, 1024], CDT, tag="exp")
                    offs = []
                    for half in (0, 1):
                        kb = kb0 + half
                        m = kb - 4 * qc
                        off = max(0, m) * P
                        offs.append(off)
                        nc.tensor.matmul(
                            ps2[:, half * 512 + off : half * 512 + 512],
                            qkT_sb[hp : hp + DH, 2 + mq, kb * P : (kb + 1) * P],
                            qkT_sb[
                                hp : hp + DH, mq, qc * 512 + off : (qc + 1) * 512
                            ],
                            start=True,
                            stop=True,
                            skip_group_check=True,
                        )
                    if offs[0] == 0 and offs[1] == 0:
                        nc.scalar.activation(exp2[:], ps2[:], EXP, scale=0.125)
                    else:
                        for half, off in enumerate(offs):
                            lo = half * 512 + off
                            nc.scalar.activation(
                                exp2[:, lo : half * 512 + 512],
                                ps2[:, lo : half * 512 + 512],
                                EXP,
                                scale=0.125,
                            )
                    for half, off in enumerate(offs):
                        kb = kb0 + half
                        if kb - 4 * qc >= 0:
                            lo = half * 512 + off
                            nc.vector.tensor_mul(
                                out=exp2[:, lo : lo + P],
                                in0=exp2[:, lo : lo + P],
                                in1=maskT_sb[:],
                            )
                        avs.append((exp2, half * 512 + off, off, kb))
                    pair_idx += 1
                    if pair_idx % 2 == 0 and fillers:
                        fillers.popleft()()
                ps_av = pp_av.tile([DH + 1, 512], F32, tag="av")
                for j, (exp2, lo, off, kb) in enumerate(avs):
                    nc.tensor.matmul(
                        ps_av[:, off:512],
                        v_sb[:, kb, h, :],
                        exp2[:, lo : (lo - off) + 512],
                        start=(j == 0),
                        stop=(j == len(avs) - 1),
                        skip_group_check=True,
                    )
                sums_sb = sb_small.tile([1, 512], F32, tag="sums")
                nc.vector.tensor_copy(out=sums_sb[:], in_=ps_av[DH : DH + 1, :])
                recip_f = sb_small.tile([1, 512], F32, tag="recipf")
                nc.vector.reciprocal_approx_fast(out=recip_f[:], in_=sums_sb[:])
                bc_sb = sb_small.tile([DH, 512], F32, tag="bc")
                nc.gpsimd.partition_broadcast(bc_sb[:], recip_f[:])
                nc.vector.tensor_mul(
                    out=attn_sb[hp : hp + DH, mq, qc * 512 : (qc + 1) * 512],
                    in0=ps_av[0:DH, :],
                    in1=bc_sb[:],
                )
                for _ in range(post):
                    if fillers:
                        fillers.popleft()()

            # ---------- schedule ----------
            from collections import deque

            # bootstrap: projections for qc0
            for slot in range(4):
                qk_chain(0, slot)
            for j in range(4):
                v_chain(0, j)

            def qk_u(qc, s):
                return lambda: qk_chain(qc, s)

            def v_u(qc, j):
                return lambda: v_chain(qc, j)

            def o_u(sc, en):
                return lambda: outproj_unit(sc, en)

            def outs(*scs):
                return [o_u(sc, en) for sc in scs for en in range(2)]

            # (qc, h, fillers): heads of qc2/qc3 interleaved so the last
            # head's exps (which gate the tail) start earlier.
            sched = [
                (0, 0, [qk_u(1, 0), qk_u(1, 1)]),
                (0, 1, [qk_u(1, 2), qk_u(1, 3), qk_u(2, 0)]),
                (0, 2, [v_u(1, 0), v_u(1, 1), qk_u(2, 1)]),
                (0, 3, [v_u(1, 2), v_u(1, 3), qk_u(2, 2)]),
                (1, 0, [qk_u(2, 3), v_u(2, 0), v_u(2, 1)]),
                (1, 1, [v_u(2, 2), v_u(2, 3), qk_u(3, 0)]),
                (1, 2, outs(0, 1)),
                (1, 3, outs(2, 3)),
                (2, 0, [qk_u(3, 1), qk_u(3, 2), qk_u(3, 3)]),
                (2, 1, [v_u(3, j) for j in range(4)]),
                (3, 0, outs(4, 5)),
                (2, 2, outs(6, 7)),
                (3, 1, []),
                (2, 3, []),
                (3, 2, outs(8, 9)),
                (3, 3, outs(10, 11)),
            ]
            for qc, h, fl in sched:
                attention_head(qc, h, deque(fl))
            # tail: outproj for q-chunks 12..15
            for sc in range(12, 16):
                for en in range(2):
                    outproj_unit(sc, en, tail=True)

    nc.compile()
    return nc


def _get_nc():
    if "nc" not in _cache:
        _cache["nc"] = _build()
    return _cache["nc"]


def _shard(x, mask, Wqkv, Wo):
    import ml_dtypes

    cdt = ml_dtypes.bfloat16
    in_maps = []
    maskT = np.ascontiguousarray((mask[0, 0, :P, :P].T >= 0).astype(cdt))
    for c in range(NCORES):
        b = c // 4
        g = c % 4
        heads = [4 * g + i for i in range(HPC)]
        q_rows = np.concatenate([np.arange(h * DH, (h + 1) * DH) for h in heads])
        k_rows = D + q_rows
        v_rows = 2 * D + q_rows
        qk_rows = np.concatenate([q_rows, k_rows])
        in_maps.append(
            {
                "xT": np.ascontiguousarray(x[b].T.astype(cdt)),
                "xT8": np.ascontiguousarray(x[b].T.astype(ml_dtypes.float8_e4m3)),
                "wqkT": np.ascontiguousarray(
                    Wqkv[qk_rows, :].T.astype(ml_dtypes.float8_e4m3)
                ),
                "wvT": np.ascontiguousarray(Wqkv[v_rows, :].T.astype(cdt)),
                "woT": np.ascontiguousarray(Wo[:, q_rows].T.astype(cdt)),
                "maskT": maskT,
            }
        )
    return in_maps


def kernel(x, mask, Wqkv, Wo, _trace=False):
    from concourse.bass_utils import run_bass_kernel_spmd

    x = np.asarray(x, dtype=np.float32)
    mask = np.asarray(mask, dtype=np.float32)
    Wqkv = np.asarray(Wqkv, dtype=np.float32)
    Wo = np.asarray(Wo, dtype=np.float32)

    nc = _get_nc()
    in_maps = _shard(x, mask, Wqkv, Wo)
    res = run_bass_kernel_spmd(nc, in_maps, core_ids=list(range(NCORES)), trace=_trace)
    _cache["last_result"] = res

    out = np.zeros((B, S, D), dtype=np.float32)
    for c in range(NCORES):
        out[c // 4] += np.asarray(res.results[c]["out"], dtype=np.float32)
    return out
